# revision 1
# baseline (speedup 1.0000x reference)
"""L2-distance self-attention (B=2, N=2048, D=1024, H=16) on 8 trn2 NeuronCores.

Sharding: core c handles batch c//4 and heads 4*(c%4) .. 4*(c%4)+4.
Each core computes its 4 heads end-to-end (q/k/v projection, L2 softmax
attention, per-head output projection partial) and returns a (2048, 1024)
fp32 partial of the output projection; the host sums the 4 partials per
batch and adds bo.

Math per core (hd = 64, heads h = 0..3):
  qb = x @ wq_h.T + bq_h            (computed transposed: (64, N); bias via
                                     ones-row augmentation of the K dim)
  kb2 = -2*(x @ wk_h.T + bk_h)      (host pre-scales wk, bk by -2)
  d2[j,i] = q2[i] + k2[j] - 2*qk    via one K=66 matmul:
     lhsT = k_stat rows [kb2 (64); ones; k2],  rhs = q_aug rows [qb (64); q2; ones]
  s = sqrt(d2)   (ACT, PSUM->SBUF fp16; exp(-s) needs no max-subtract since s>=0)
  e = exp(-s)    (ACT, strided i-chunk read so PV can consume i-chunk major)
  oT_aug = v_aug.T @ e  with v_aug = [v | ones] -> row 64 = softmax denominator
  y_h = oT_h.T @ woT_h ; y_acc += y_h * (1/den[i])   (DVE fused multiply-add)
"""

import sys

for p in ("/opt/trn_rl_repo", "/root/.axon_site/_ro/trn_rl_repo"):
    if p not in sys.path:
        sys.path.append(p)

import numpy as np

B, N, D, H = 2, 2048, 1024, 16
HD = 64          # head dim
HPC = 4          # heads per core
HS = HPC * HD    # head-group width per core (256)
NB = N // 128    # 16 j/i blocks
IC = N // 512    # 4 projection moving chunks
KB = D // 128    # 8 contraction blocks for projections
EC = 256         # exp/PV i-chunk width
NEC = N // EC    # 8 exp chunks per head

_CACHE = {}


def _build(loop_n=None):
    import concourse.bacc as bacc
    import concourse.mybir as mybir
    import concourse.tile as tile

    dt = mybir.dt
    AF = mybir.ActivationFunctionType
    ALU = mybir.AluOpType

    nc = bacc.Bacc("TRN2", target_bir_lowering=False, debug=False)

    # ---- DRAM I/O (per core) ----
    xT = nc.dram_tensor("xT", [D, N], dt.float16, kind="ExternalInput")
    wq = nc.dram_tensor("wq_aug", [D + 1, HS], dt.float16, kind="ExternalInput")
    wk = nc.dram_tensor("wk_aug", [D + 1, HS], dt.float16, kind="ExternalInput")
    wv = nc.dram_tensor("wv_aug", [D + 1, HS], dt.float16, kind="ExternalInput")
    wo = nc.dram_tensor("woT", [HS, D], dt.float16, kind="ExternalInput")
    y = nc.dram_tensor("y", [N, D], dt.float32, kind="ExternalOutput")

    with tile.TileContext(nc) as tc:
        with (
            tc.tile_pool(name="cst", bufs=1) as cst,
            tc.tile_pool(name="u4", bufs=9) as u4,      # 4KB slots: xt -> oT -> y
            tc.tile_pool(name="wp", bufs=1) as wp,
            tc.tile_pool(name="wop", bufs=1) as wop,
            tc.tile_pool(name="aug", bufs=1) as aug,
            tc.tile_pool(name="dp", bufs=1) as dpool,
            tc.tile_pool(name="spool", bufs=1) as spool,
            tc.tile_pool(name="e8", bufs=2) as e8,      # 8KB slots: sqtmp / e
            tc.tile_pool(name="psum", bufs=2, space="PSUM") as ps,
        ):
            # ---- constants ----
            ones_row = cst.tile([1, 512], dt.float16, tag="ones_row")
            nc.gpsimd.memset(ones_row[:], 1.0)
            # E66 matrices: ones-matmul producers for the two augmentation
            # rows (64, 65) of q_aug / k_stat. q_aug: row 64 = q2, row 65 = 1.
            # k_stat: row 64 = 1, row 65 = 0.25*sum(kb2^2) = k2.
            e66q = cst.tile([64, 66], dt.float16, tag="e66q")
            nc.gpsimd.memset(e66q[:], 0.0)
            nc.gpsimd.memset(e66q[:, 64:65], 1.0)
            e66k = cst.tile([64, 66], dt.float16, tag="e66k")
            nc.gpsimd.memset(e66k[:], 0.0)
            nc.gpsimd.memset(e66k[:, 65:66], 0.25)
            e66aq = cst.tile([1, 66], dt.float16, tag="e66aq")  # ones into row 65
            nc.gpsimd.memset(e66aq[:], 0.0)
            nc.gpsimd.memset(e66aq[:, 65:66], 1.0)
            e66ak = cst.tile([1, 66], dt.float16, tag="e66ak")  # ones into row 64
            nc.gpsimd.memset(e66ak[:], 0.0)
            nc.gpsimd.memset(e66ak[:, 64:65], 1.0)

            # PE warmup: dependency-free matmuls keep the tensor engine busy
            # through the input-DMA window so real matmuls start at full clock
            for w in range(2):
                wup = ps.tile([128, 512], dt.float32, tag="big", name="wup")
                for r in range(12):
                    nc.tensor.matmul(
                        wup[:], ones_row[0:1, 0:128], ones_row[0:1, :],
                        start=(r == 0), stop=(r == 11),
                    )

            # ---- load inputs (order tuned so q-proj ic0 can start ASAP;
            # wv/wo go through the gpsimd DGE queue to parallelize issue) ----
            xt = [u4.tile([128, N], dt.float16, tag="u4", name=f"xt{k}") for k in range(KB)]
            wq_all = wp.tile([128, KB * HS], dt.float16, tag="wq_all")
            wk_all = wp.tile([128, KB * HS], dt.float16, tag="wk_all")
            wv_all = wp.tile([128, KB * HS], dt.float16, tag="wv_all")
            brows = wp.tile([1, 3 * HS], dt.float16, tag="brows")
            nc.sync.dma_start(brows[:, 0:HS], wq[D : D + 1, :])
            nc.sync.dma_start(brows[:, HS : 2 * HS], wk[D : D + 1, :])
            for k in range(KB):
                nc.sync.dma_start(
                    wq_all[:, k * HS : (k + 1) * HS], wq[k * 128 : (k + 1) * 128, :]
                )
                nc.sync.dma_start(xt[k][:], xT[k * 128 : (k + 1) * 128, :])
            for k in range(KB):
                nc.sync.dma_start(
                    wk_all[:, k * HS : (k + 1) * HS], wk[k * 128 : (k + 1) * 128, :]
                )
            nc.gpsimd.dma_start(brows[:, 2 * HS : 3 * HS], wv[D : D + 1, :])
            for k in range(KB):
                nc.gpsimd.dma_start(
                    wv_all[:, k * HS : (k + 1) * HS], wv[k * 128 : (k + 1) * 128, :]
                )

            # ---- per-head augmented tiles ----
            q_aug = [aug.tile([66, N], dt.float16, tag=f"qa{h}", name=f"qa{h}") for h in range(HPC)]
            k_stat = [aug.tile([66, N], dt.float16, tag=f"ks{h}", name=f"ks{h}") for h in range(HPC)]
            # v tiles: per jb, 4 head-blocks of [v(64) | ones]
            v_jb = [aug.tile([128, HPC * 65], dt.float16, tag=f"v{jb}", name=f"v{jb}") for jb in range(NB)]
            for jb in range(NB):
                nc.gpsimd.memset(
                    v_jb[jb][:].rearrange("p (b d) -> p b d", d=65)[:, :, 64:65], 1.0
                )

            # ---- q/k projections (Mblk = head pair), bias via extra K row ----
            def proj_qk_ic(w_all, boff, dest, m, ic):
                # heads 2m, 2m+1 ; psum (128 d, 512 i), one i-chunk
                p = ps.tile([128, 512], dt.float32, tag="big", name="pp")
                for k in range(KB + 1):
                    if k < KB:
                        lhsT = w_all[:, k * HS + m * 128 : k * HS + (m + 1) * 128]
                        rhs = xt[k][:, ic * 512 : (ic + 1) * 512]
                    else:
                        lhsT = brows[0:1, boff + m * 128 : boff + (m + 1) * 128]
                        rhs = ones_row[0:1, :]
                    nc.tensor.matmul(p[:], lhsT, rhs, start=(k == 0), stop=(k == KB))
                for half in range(2):
                    nc.vector.tensor_copy(
                        dest[2 * m + half][0:64, ic * 512 : (ic + 1) * 512],
                        p[64 * half : 64 * half + 64, :],
                    )

            # q2 (row 64 of q_aug, row 65 ones) / k2 (row 65 of k_stat, row 64 ones)
            def norms_part(h, which):
                src_tile = q_aug[h] if which == "q" else k_stat[h]
                emat = e66q if which == "q" else e66k
                eadd = e66aq if which == "q" else e66ak
                sq = u4.tile([64, N], dt.float16, tag="u4", name="sq")
                nc.vector.tensor_tensor(
                    out=sq[:], in0=src_tile[0:64, :], in1=src_tile[0:64, :],
                    op=ALU.mult,
                )
                p = ps.tile([66, N], dt.float32, tag="big", name="np")
                for ic in range(IC):
                    nc.tensor.matmul(
                        p[:, ic * 512 : (ic + 1) * 512], emat[:],
                        sq[:, ic * 512 : (ic + 1) * 512], start=True, stop=False,
                    )
                    nc.tensor.matmul(
                        p[:, ic * 512 : (ic + 1) * 512], eadd[:],
                        ones_row[0:1, :], start=False, stop=True,
                    )
                nc.vector.tensor_copy(src_tile[64:66, :], p[64:66, :])

            def vp_one(jb):
                p = ps.tile([128, HS], dt.float32, tag="big", name="vp")
                for k in range(KB + 1):
                    if k < KB:
                        lhsT = xt[k][:, jb * 128 : (jb + 1) * 128]
                        rhs = wv_all[:, k * HS : (k + 1) * HS]
                    else:
                        lhsT = ones_row[0:1, 0:128]
                        rhs = brows[0:1, 2 * HS : 3 * HS]
                    nc.tensor.matmul(p[:], lhsT, rhs, start=(k == 0), stop=(k == KB))
                dst = v_jb[jb][:].rearrange("p (h d) -> p h d", d=65)[:, :, 0:64]
                nc.vector.tensor_copy(dst, p[:].rearrange("p (h d) -> p h d", d=64))

            s = spool.tile([128, NB * N], dt.float16, tag="s")
            sv = s[:].rearrange("p (t i) -> p t i", t=NB)
            oTp = [
                aug.tile([128, N], dt.float16, tag="oTp0", name="oTp0"),
                aug.tile([128, N], dt.float16, tag="oTp1", name="oTp1"),
            ]
            raws = [None] * HPC

            def st_one(h, jb):
                st = ps.tile([128, N], dt.float32, tag="big", name="st")
                for ic in range(IC):
                    nc.tensor.matmul(
                        st[:, ic * 512 : (ic + 1) * 512],
                        k_stat[h][0:66, jb * 128 : (jb + 1) * 128],
                        q_aug[h][0:66, ic * 512 : (ic + 1) * 512],
                        start=True, stop=True,
                    )
                nc.scalar.activation(s[:, jb * N : (jb + 1) * N], st[:], AF.Sqrt)

            def exp_chunk(h, c, pv):
                e = e8.tile([128, NB * EC], dt.float16, tag="e8", name="e")
                nc.scalar.activation(
                    e[:].rearrange("p (t i) -> p t i", t=NB),
                    sv[:, :, c * EC : (c + 1) * EC],
                    AF.Exp, scale=-1.0,
                )
                for t in range(NB):
                    nc.tensor.matmul(
                        pv[:, c * EC : (c + 1) * EC],
                        v_jb[t][:, h * 65 : h * 65 + 65],
                        e[:, t * EC : (t + 1) * EC],
                        start=(t == 0), stop=(t == NB - 1),
                    )

            def raw_copy(h, pv):
                raws[h] = u4.tile([65, N], dt.float16, tag="u4", name=f"raw{h}")
                with nc.allow_low_precision(reason="fp16 softmax weights"):
                    nc.vector.tensor_copy(raws[h][:], pv[:])
                    nc.vector.reciprocal(out=dpool_row[:], in_=raws[h][64:65, :])

            def norm_bc_mm(h):
                bc = ps.tile([64, N], dt.float32, tag="big", name="bc")
                for ic in range(IC):
                    nc.tensor.matmul(
                        bc[:, ic * 512 : (ic + 1) * 512],
                        ones_row[0:1, 0:64],
                        dpool_row[0:1, ic * 512 : (ic + 1) * 512],
                        start=True, stop=True,
                    )
                return bc

            def norm_finish(h, bc):
                bcs = u4.tile([64, N], dt.float16, tag="u4", name=f"bcs{h}")
                nc.vector.tensor_copy(bcs[:], bc[:])
                half = 64 * (h % 2)
                nc.vector.tensor_tensor(
                    out=oTp[h // 2][half : half + 64, :],
                    in0=raws[h][0:64, :], in1=bcs[:], op=ALU.mult,
                )

            dpool_row = dpool.tile([1, N], dt.float16, tag="dinrow")

            # ================= emission schedule =================
            # lead-in: head-pair-0 q/k projections + head-0 norms
            for ic in range(IC):
                proj_qk_ic(wq_all, 0, q_aug, 0, ic)
            for ic in range(IC):
                proj_qk_ic(wk_all, HS, k_stat, 0, ic)
            norms_part(0, "q")
            norms_part(0, "k")

            # head-0 S.T/sqrt; v-proj + head-1 norms fill PE gaps
            st_one(0, 0)
            norms_part(1, "q")
            st_one(0, 1)
            norms_part(1, "k")
            for jb in range(2, NB):
                st_one(0, jb)
                vp_one(jb - 2)
            vp_one(NB - 2)
            vp_one(NB - 1)

            # wo loads (no psum; DMA only)
            wotp = [wop.tile([128, D], dt.float16, tag=f"wop{p}", name=f"wop{p}") for p in range(2)]
            for p in range(2):
                nc.gpsimd.dma_start(wotp[p][:], wo[p * 128 : (p + 1) * 128, :])

            # head-0 exp/PV; head-pair-1 projections fill PE gaps
            m1 = [(wq_all, 0, q_aug), (wk_all, HS, k_stat)]
            pv = ps.tile([65, N], dt.float32, tag="big", name="pv")
            for c in range(NEC):
                exp_chunk(0, c, pv)
                w_all_, boff_, dest_ = m1[c // IC]
                proj_qk_ic(w_all_, boff_, dest_, 1, c % IC)
            raw_copy(0, pv)

            # head-1 S.T/sqrt; normalize-0 + head-2/3 norms fill gaps
            st_one(1, 0)
            st_one(1, 1)
            st_one(1, 2)
            bc0 = norm_bc_mm(0)
            st_one(1, 3)
            norm_finish(0, bc0)
            for jb in range(4, NB):
                st_one(1, jb)
                if jb == 4:
                    norms_part(2, "q")
                elif jb == 6:
                    norms_part(2, "k")
                elif jb == 8:
                    norms_part(3, "q")
                elif jb == 10:
                    norms_part(3, "k")

            for h in range(1, HPC - 1):
                pv = ps.tile([65, N], dt.float32, tag="big", name="pv")
                for c in range(NEC):
                    exp_chunk(h, c, pv)
                raw_copy(h, pv)
                st_one(h + 1, 0)
                st_one(h + 1, 1)
                st_one(h + 1, 2)
                bc = norm_bc_mm(h)
                st_one(h + 1, 3)
                norm_finish(h, bc)
                for jb in range(4, NB):
                    st_one(h + 1, jb)

            # last head: normalize chunk-wise right behind each PV chunk so
            # oT is complete almost as soon as the last exp finishes
            pv = ps.tile([65, N], dt.float32, tag="big", name="pv")
            bc = ps.tile([64, N], dt.float32, tag="big", name="bc3")
            h = HPC - 1
            for c in range(NEC):
                exp_chunk(h, c, pv)
                lo, hi = c * EC, (c + 1) * EC
                with nc.allow_low_precision(reason="fp16 softmax weights"):
                    nc.vector.reciprocal(
                        out=dpool_row[0:1, lo:hi], in_=pv[64:65, lo:hi]
                    )
                nc.tensor.matmul(
                    bc[:, lo:hi], ones_row[0:1, 0:64], dpool_row[0:1, lo:hi],
                    start=True, stop=True,
                )
                bcs = u4.tile([64, EC], dt.float16, tag="u4", name="bcs3c")
                nc.vector.tensor_copy(bcs[:], bc[:, lo:hi])
                with nc.allow_low_precision(reason="fp16 softmax weights"):
                    nc.vector.tensor_tensor(
                        out=oTp[h // 2][64 : 128, lo:hi],
                        in0=pv[0:64, lo:hi], in1=bcs[:], op=ALU.mult,
                    )

            # ---- output projection: all heads accumulate in PSUM ----
            def yp_mms(yp, ib, pairs, stop_p):
                for pr in pairs:
                    for fc in range(2):
                        nc.tensor.matmul(
                            yp[:, fc * 512 : (fc + 1) * 512],
                            oTp[pr][:, ib * 128 : (ib + 1) * 128],
                            wotp[pr][:, fc * 512 : (fc + 1) * 512],
                            start=(pr == 0), stop=(pr == stop_p),
                        )

            def yac_out(yp, ib):
                yac = u4.tile([128, D], dt.float32, tag="u4", name="yac")
                if ib % 2 == 0:
                    nc.scalar.copy(yac[:], yp[:])
                else:
                    nc.vector.tensor_copy(yac[:], yp[:])
                nc.sync.dma_start(y[ib * 128 : (ib + 1) * 128, :], yac[:])

            yp0 = ps.tile([128, D], dt.float32, tag="big", name="yp")
            yp_mms(yp0, 0, [0], 1)
            yp1 = ps.tile([128, D], dt.float32, tag="big", name="yp")
            yp_mms(yp1, 1, [0], 1)
            yp_mms(yp0, 0, [1], 1)
            yac_out(yp0, 0)
            yp_mms(yp1, 1, [1], 1)
            yac_out(yp1, 1)
            for ib in range(2, NB):
                yp = ps.tile([128, D], dt.float32, tag="big", name="yp")
                yp_mms(yp, ib, [0, 1], 1)
                yac_out(yp, ib)

    nc.compile()
    return nc


def _prep_in_maps(x, wq, bq, wk, bk, wv, bv, wo):
    f16 = np.float16
    in_maps = []
    xTs = [np.ascontiguousarray(x[b].T).astype(f16) for b in range(B)]
    for c in range(8):
        b, hg = divmod(c, HPC)
        hs = hg * HS
        wq_aug = np.concatenate(
            [wq[hs : hs + HS, :].T, bq[None, hs : hs + HS]], axis=0
        ).astype(f16)
        wk_aug = np.concatenate(
            [-2.0 * wk[hs : hs + HS, :].T, -2.0 * bk[None, hs : hs + HS]], axis=0
        ).astype(f16)
        wv_aug = np.concatenate(
            [wv[hs : hs + HS, :].T, bv[None, hs : hs + HS]], axis=0
        ).astype(f16)
        woT = np.ascontiguousarray(wo[:, hs : hs + HS].T).astype(f16)
        in_maps.append(
            {
                "xT": xTs[b],
                "wq_aug": np.ascontiguousarray(wq_aug),
                "wk_aug": np.ascontiguousarray(wk_aug),
                "wv_aug": np.ascontiguousarray(wv_aug),
                "woT": woT,
            }
        )
    return in_maps


def _get_nc():
    if "nc" not in _CACHE:
        _CACHE["nc"] = _build()
    return _CACHE["nc"]


def run(inputs, trace=False, **trace_kwargs):
    """Run on 8 cores; returns (full_output, BassKernelResults)."""
    from concourse.bass_utils import run_bass_kernel_spmd

    nc = _get_nc()
    in_maps = _prep_in_maps(
        np.asarray(inputs["x"], np.float32),
        np.asarray(inputs["wq"], np.float32), np.asarray(inputs["bq"], np.float32),
        np.asarray(inputs["wk"], np.float32), np.asarray(inputs["bk"], np.float32),
        np.asarray(inputs["wv"], np.float32), np.asarray(inputs["bv"], np.float32),
        np.asarray(inputs["wo"], np.float32),
    )
    res = run_bass_kernel_spmd(nc, in_maps, list(range(8)), trace=trace, **trace_kwargs)
    bo = np.asarray(inputs["bo"], np.float32)
    out = np.empty((B, N, D), np.float32)
    for b in range(B):
        acc = res.results[b * HPC]["y"].astype(np.float32)
        for c in range(b * HPC + 1, (b + 1) * HPC):
            acc = acc + res.results[c]["y"]
        out[b] = acc + bo
    return out, res


def kernel(**inputs) -> np.ndarray:
    out, _ = run(inputs, trace=False)
    return out


if __name__ == "__main__":
    rng = np.random.default_rng(0)
    ins = {
        "x": rng.standard_normal((B, N, D)).astype(np.float32),
        "wq": (rng.standard_normal((D, D)) * 0.02).astype(np.float32),
        "bq": (rng.standard_normal(D) * 0.02).astype(np.float32),
        "wk": (rng.standard_normal((D, D)) * 0.02).astype(np.float32),
        "bk": (rng.standard_normal(D) * 0.02).astype(np.float32),
        "wv": (rng.standard_normal((D, D)) * 0.02).astype(np.float32),
        "bv": (rng.standard_normal(D) * 0.02).astype(np.float32),
        "wo": (rng.standard_normal((D, D)) * 0.02).astype(np.float32),
        "bo": (rng.standard_normal(D) * 0.02).astype(np.float32),
    }
    print(kernel(**ins).shape)



# revision 16
# speedup vs baseline: 1.0990x; 1.0990x over previous
"""L2-distance self-attention (B=2, N=2048, D=1024, H=16) on 8 trn2 NeuronCores.

Sharding: core c handles batch c//4 and heads 4*(c%4) .. 4*(c%4)+4.
Each core computes its 4 heads end-to-end and returns TWO (2048, 1024) fp32
partials of the output projection (head pair 0 and head pair 1); the host
sums the 8 partials per batch and adds bo_eff = bo + wo @ bv (the v-bias
contributes exactly wo@bv after softmax normalization, so it is folded out
of the device kernel).

Per-head pipeline (ACT is the bottleneck engine at ~60us/head):
  sqrt phase: PE emits d2 = q2[i] + k2[j] - 2qk via K=66 matmuls (two
    512-wide halves per j-block into 2-bank psum slots). j-blocks 0-3 are
    consumed by ACT Sqrt directly from PSUM; j-blocks 4-15 are drained by
    DVE casts (fp32->fp16) into the s tile and ACT runs Sqrt in-place over
    4-block groups. PE/DVE stay ahead of ACT's 1.78us/block pace.
  exp phase: ACT Exp (scale=-1) over strided i-chunks; PE runs PV matmuls
    (v_aug with a ones column -> row 64 = softmax denominator) plus the
    first j-block of the NEXT head's d2, so the next sqrt phase starts hot.
  normalize: denominator row -> fp32, reciprocal_approx_fast (DVE custom
    op), PE broadcast matmul, DVE multiply into oTp -- per i-half, placed
    in DVE-idle exp phases.
  out-proj: pair-0 y during heads 2/3 sqrt phases, pair-1 y at the tail;
    each pair goes to its own DRAM tensor.
"""

import sys

for p in ("/opt/trn_rl_repo", "/root/.axon_site/_ro/trn_rl_repo"):
    if p not in sys.path:
        sys.path.append(p)

import numpy as np

B, N, D, H = 2, 2048, 1024, 16
HD = 64          # head dim
HPC = 4          # heads per core
HS = HPC * HD    # head-group width per core (256)
NB = N // 128    # 16 j-blocks
IC = N // 512    # 4 i-chunks of 512
KB = D // 128    # 8 contraction blocks for projections
EC = 256         # exp/PV i-chunk width
NEC = N // EC    # 8 exp chunks per head

_CACHE = {}


def _build():
    import concourse.bacc as bacc
    import concourse.mybir as mybir
    import concourse.tile as tile

    dt = mybir.dt
    AF = mybir.ActivationFunctionType
    ALU = mybir.AluOpType

    nc = bacc.Bacc("TRN2", target_bir_lowering=False, debug=False)

    # ---- DRAM I/O (per core) ----
    xT = nc.dram_tensor("xT", [D, N], dt.float16, kind="ExternalInput")
    wq = nc.dram_tensor("wq_t", [D, HS], dt.float16, kind="ExternalInput")
    wk = nc.dram_tensor("wk_t", [D, HS], dt.float16, kind="ExternalInput")
    wv = nc.dram_tensor("wv_t", [D, HS], dt.float16, kind="ExternalInput")
    wo = nc.dram_tensor("woT", [HS, D], dt.float16, kind="ExternalInput")
    bias_d = nc.dram_tensor("biases", [128, 4], dt.float32, kind="ExternalInput")
    y0 = nc.dram_tensor("y0", [N, D], dt.float32, kind="ExternalOutput")
    y1 = nc.dram_tensor("y1", [N, D], dt.float32, kind="ExternalOutput")
    ydram = [y0, y1]

    with tile.TileContext(nc) as tc:
        with (
            tc.tile_pool(name="cst", bufs=1) as cst,
            tc.tile_pool(name="u4", bufs=9) as u4,        # 4KB slots: xt, sq, yac
            tc.tile_pool(name="wp", bufs=1) as wp,
            tc.tile_pool(name="wop", bufs=1) as wop,
            tc.tile_pool(name="aug", bufs=1) as aug,
            tc.tile_pool(name="rawp", bufs=2) as rawp,    # raws[h] rotate
            tc.tile_pool(name="dp", bufs=1) as dpool,
            tc.tile_pool(name="spool", bufs=1) as spool,
            tc.tile_pool(name="e8", bufs=2) as e8,
            tc.tile_pool(name="psum", bufs=2, space="PSUM") as ps,
        ):
            # ---- constants ----
            ones_row = cst.tile([1, 512], dt.float16, tag="ones_row")
            nc.gpsimd.memset(ones_row[:], 1.0)
            ones64f = cst.tile([1, 64], dt.float32, tag="ones64f")
            nc.gpsimd.memset(ones64f[:], 1.0)
            # column constants for the q2 (ones) / k2 (0.25) reductions
            onescol = cst.tile([128, 1], dt.float16, tag="onescol")
            nc.gpsimd.memset(onescol[:], 1.0)
            qcol = cst.tile([128, 1], dt.float16, tag="qcol")
            nc.gpsimd.memset(qcol[:], 0.25)

            bias_pp = cst.tile([128, 4], dt.float32, tag="bias_pp")
            nc.sync.dma_start(bias_pp[:], bias_d[:, :])

            # ---- per-head augmented tiles ----
            # q_aug rows: [qb (0-63); q2 (64)]; k_stat rows: [kb2 (0-63);
            # ones (64)].  ST K=65: row 64 contributes 1*q2[i]; the k2[j]
            # term is applied as the per-partition bias of the Sqrt (psum-
            # direct blocks) or in the DVE drain (tensor_scalar_add).
            q_aug = [aug.tile([66, N], dt.float16, tag=f"qa{h}", name=f"qa{h}") for h in range(HPC)]
            k_stat = [aug.tile([66, N], dt.float16, tag=f"ks{h}", name=f"ks{h}") for h in range(HPC)]
            for h in range(HPC):
                nc.gpsimd.memset(k_stat[h][64:66, :], 1.0)
            # k2 in column layout: k2c[h][j%128, jb] for j-block jb
            k2c = [cst.tile([128, NB], dt.float32, tag=f"k2c{h}", name=f"k2c{h}") for h in range(HPC)]
            v_jb = [aug.tile([128, HPC * 65], dt.float16, tag=f"v{jb}", name=f"v{jb}") for jb in range(NB)]
            for jb in range(NB):
                nc.gpsimd.memset(
                    v_jb[jb][:].rearrange("p (b d) -> p b d", d=65)[:, :, 64:65], 1.0
                )
            oTp = [
                aug.tile([128, N], dt.float16, tag="oTp0", name="oTp0"),
                aug.tile([128, N], dt.float16, tag="oTp1", name="oTp1"),
            ]

            # PE warmup: dependency-free matmuls release the HAM clock gate
            wup = ps.tile([128, 512], dt.float32, tag="pA", name="wup")
            for r in range(12):
                nc.tensor.matmul(
                    wup[:], ones_row[0:1, 0:128], ones_row[0:1, :],
                    start=(r == 0), stop=(r == 11),
                )

            # ---- input DMA ----
            xt = [u4.tile([128, N], dt.float16, tag="u4", name=f"xt{k}") for k in range(KB)]
            wq_all = wp.tile([128, KB * HS], dt.float16, tag="wq_all")
            wk_all = wp.tile([128, KB * HS], dt.float16, tag="wk_all")
            wv_all = wp.tile([128, KB * HS], dt.float16, tag="wv_all")
            for k in range(KB):
                nc.sync.dma_start(
                    wq_all[:, k * HS : (k + 1) * HS], wq[k * 128 : (k + 1) * 128, :]
                )
                nc.sync.dma_start(xt[k][:], xT[k * 128 : (k + 1) * 128, :])
            for k in range(KB):
                nc.sync.dma_start(
                    wk_all[:, k * HS : (k + 1) * HS], wk[k * 128 : (k + 1) * 128, :]
                )
            for k in range(KB):
                nc.gpsimd.dma_start(
                    wv_all[:, k * HS : (k + 1) * HS], wv[k * 128 : (k + 1) * 128, :]
                )
            wotp = [wop.tile([128, D], dt.float16, tag=f"wop{p}", name=f"wop{p}") for p in range(2)]
            for p in range(2):
                nc.gpsimd.dma_start(wotp[p][:], wo[p * 128 : (p + 1) * 128, :])

            # ---- big SBUF tiles ----
            s = spool.tile([128, NB * N], dt.float16, tag="s")
            sv = s[:].rearrange("p (t i) -> p t i", t=NB)

            raws = [None] * HPC
            pend_st = {}   # (h, jb) -> (tileA, tileB) kept in PSUM
            pend_pv = {}   # h -> pvB psum tile (den row consumed next phase)

            ic_sl = lambda ic: slice(ic * 512, (ic + 1) * 512)

            # ---- helpers (emission) ----
            def proj_qk_ic(w_all, bcol, dest, m, ic, tag):
                # heads 2m, 2m+1; psum (128 d, 512 i); bias fused in copy
                p = ps.tile([128, 512], dt.float32, tag=tag, name="pp")
                for k in range(KB):
                    nc.tensor.matmul(
                        p[:],
                        w_all[:, k * HS + m * 128 : k * HS + (m + 1) * 128],
                        xt[k][:, ic_sl(ic)],
                        start=(k == 0), stop=(k == KB - 1),
                    )
                for half in range(2):
                    with nc.allow_low_precision(reason="fp16 activations"):
                        nc.vector.tensor_scalar_add(
                            out=dest[2 * m + half][0:64, ic_sl(ic)],
                            in0=p[64 * half : 64 * half + 64, :],
                            scalar1=bias_pp[64 * half : 64 * half + 64, bcol + m : bcol + m + 1],
                        )

            def norms(h):
                # q2 into q_aug[h] row 64 (row layout); k2 into k2c[h]
                # (column layout -- no transpose needed: contract over d with
                # output partitions = j)
                sq = u4.tile([128, N], dt.float16, tag="u4", name="sq")
                nc.vector.tensor_tensor(
                    out=sq[0:64, :], in0=q_aug[h][0:64, :], in1=q_aug[h][0:64, :],
                    op=ALU.mult,
                )
                nc.vector.tensor_tensor(
                    out=sq[64:128, :], in0=k_stat[h][0:64, :], in1=k_stat[h][0:64, :],
                    op=ALU.mult,
                )
                for half in range(2):
                    p = ps.tile([1, 1024], dt.float32, tag="pA" if half == 0 else "pB", name="np")
                    for d_ in range(2):
                        ic = 2 * half + d_
                        nc.tensor.matmul(
                            p[:, d_ * 512 : (d_ + 1) * 512],
                            onescol[0:64, :], sq[0:64, ic_sl(ic)],
                            start=True, stop=True,
                        )
                    with nc.allow_low_precision(reason="fp16 stats"):
                        nc.vector.tensor_copy(
                            q_aug[h][64:65, half * 1024 : (half + 1) * 1024], p[:]
                        )
                k2p = ps.tile([128, NB], dt.float32, tag="pA", name="k2p")
                for jb in range(NB):
                    nc.tensor.matmul(
                        k2p[:, jb : jb + 1],
                        sq[64:128, jb * 128 : (jb + 1) * 128],
                        qcol[64:128, :],
                        start=True, stop=True,
                    )
                # +eps guards sqrt against tiny-negative d2 from fp16/fp22
                # rounding when q_i ~ k_j (true d2 ~ 0)
                nc.vector.tensor_scalar_add(out=k2c[h][:], in0=k2p[:], scalar1=0.05)

            def vp_one(jb):
                p = ps.tile([128, HS], dt.float32, tag="pA" if jb % 2 == 0 else "pB", name="vp")
                for k in range(KB):
                    nc.tensor.matmul(
                        p[:], xt[k][:, jb * 128 : (jb + 1) * 128],
                        wv_all[:, k * HS : (k + 1) * HS],
                        start=(k == 0), stop=(k == KB - 1),
                    )
                dst = v_jb[jb][:].rearrange("p (h d) -> p h d", d=65)[:, :, 0:64]
                nc.vector.tensor_copy(dst, p[:].rearrange("p (h d) -> p h d", d=64))

            def st_halves(h, jb):
                # d2 (minus the k2[j] term) for j-block jb, both i-halves,
                # kept in PSUM (returned)
                ts = []
                for half, tag in ((0, "pA"), (1, "pB")):
                    t_ = ps.tile([128, 1024], dt.float32, tag=tag, name=f"st{half}")
                    for d_ in range(2):
                        ic = 2 * half + d_
                        nc.tensor.matmul(
                            t_[:, d_ * 512 : (d_ + 1) * 512],
                            k_stat[h][0:65, jb * 128 : (jb + 1) * 128],
                            q_aug[h][0:65, ic_sl(ic)],
                            start=True, stop=True,
                        )
                    ts.append(t_)
                return ts

            def st_cast(h, jb, ts):
                # drain d2 psum -> s (fp16), adding the k2[j] term; sqrt
                # later runs in-place over s
                for half in (0, 1):
                    with nc.allow_low_precision(reason="fp16 d2"):
                        nc.vector.tensor_scalar(
                            out=s[:, jb * N + half * 1024 : jb * N + (half + 1) * 1024],
                            in0=ts[half][:],
                            scalar1=k2c[h][:, jb : jb + 1],
                            scalar2=0.0,
                            op0=ALU.add,
                            op1=ALU.max,
                        )

            def sqrt_psum_direct(h, jb, ts):
                # sqrt(d2_partial + k2[j]) straight from PSUM via ACT bias
                for half in (0, 1):
                    nc.scalar.activation(
                        s[:, jb * N + half * 1024 : jb * N + (half + 1) * 1024],
                        ts[half][:], AF.Sqrt,
                        bias=k2c[h][:, jb : jb + 1],
                    )

            def sqrt_group(g):
                lo, hi = 4 * g * N, 4 * (g + 1) * N
                nc.scalar.activation(s[:, lo:hi], s[:, lo:hi], AF.Sqrt)

            def exp_chunk(h, c, pvh):
                e = e8.tile([128, NB * EC], dt.float16, tag="e8", name="e")
                nc.scalar.activation(
                    e[:].rearrange("p (t i) -> p t i", t=NB),
                    sv[:, :, c * EC : (c + 1) * EC],
                    AF.Exp, scale=-1.0,
                )
                cc = (c % 4) * EC
                for t in range(NB):
                    nc.tensor.matmul(
                        pvh[:, cc : cc + EC],
                        v_jb[t][:, h * 65 : h * 65 + 65],
                        e[:, t * EC : (t + 1) * EC],
                        start=(t == 0), stop=(t == NB - 1),
                    )

            def raw_den(h, half, pvh):
                # pv rows 0-63 -> raws fp16 (row 64 = den stays in psum)
                if half == 0:
                    raws[h] = rawp.tile([64, N], dt.float16, tag="raw", name=f"raw{h}")
                lo = half * 1024
                with nc.allow_low_precision(reason="fp16 softmax weights"):
                    nc.vector.tensor_copy(raws[h][:, lo : lo + 1024], pvh[0:64, :])

            def norm_half(h, half, pvh, tag):
                # den psum -> sbuf, dinv = 1/den (fp32 approx), broadcast via
                # PE, multiply into oTp
                lo = half * 1024
                den = dpool.tile([1, 1024], dt.float32, tag="den", bufs=2, name="den")
                nc.vector.tensor_copy(den[:], pvh[64:65, :])
                dinv = dpool.tile([1, 1024], dt.float32, tag="dinv", bufs=2, name="dinv")
                nc.vector.reciprocal_approx_fast(out=dinv[:], in_=den[:])
                bc = ps.tile([64, 1024], dt.float32, tag=tag, name="bc")
                for d_ in range(2):
                    nc.tensor.matmul(
                        bc[:, d_ * 512 : (d_ + 1) * 512],
                        ones64f[:],
                        dinv[0:1, d_ * 512 : (d_ + 1) * 512],
                        start=True, stop=True,
                    )
                row = 64 * (h % 2)
                with nc.allow_low_precision(reason="fp16 softmax weights"):
                    nc.vector.tensor_tensor(
                        out=oTp[h // 2][row : row + 64, lo : lo + 1024],
                        in0=raws[h][:, lo : lo + 1024], in1=bc[:], op=ALU.mult,
                    )

            def yout(pair, ib, tag):
                yp = ps.tile([128, D], dt.float32, tag=tag, name="yp")
                for fc in range(2):
                    nc.tensor.matmul(
                        yp[:, fc * 512 : (fc + 1) * 512],
                        oTp[pair][:, ib * 128 : (ib + 1) * 128],
                        wotp[pair][:, fc * 512 : (fc + 1) * 512],
                        start=True, stop=True,
                    )
                yac = u4.tile([128, D], dt.float32, tag="u4", name="yac")
                nc.vector.tensor_copy(yac[:], yp[:])
                nc.sync.dma_start(ydram[pair][ib * 128 : (ib + 1) * 128, :], yac[:])

            # ================= emission schedule =================
            # ---- lead-in: pair-0 projections, norms(0), first d2 block ----
            for ic in range(IC):
                proj_qk_ic(wq_all, 0, q_aug, 0, ic, "pA" if ic % 2 == 0 else "pB")
            for ic in range(IC):
                proj_qk_ic(wk_all, 2, k_stat, 0, ic, "pA" if ic % 2 == 0 else "pB")
            norms(0)
            pend_st[(0, 0)] = st_halves(0, 0)

            # ---- per-head phases ----
            for h in range(HPC):
                # ---------- sqrt phase ----------
                # j-blocks 0-3: ACT reads d2 straight from PSUM
                if h == 0:
                    pe_fills = [lambda jb=jb: vp_one(jb) for jb in range(NB)]
                    pe_fills.append(lambda: norms(1))
                elif h == 1:
                    pe_fills = [
                        lambda ic=ic: proj_qk_ic(wq_all, 0, q_aug, 1, ic, "pA" if ic % 2 == 0 else "pB")
                        for ic in range(IC)
                    ]
                    pe_fills += [
                        lambda ic=ic: proj_qk_ic(wk_all, 2, k_stat, 1, ic, "pA" if ic % 2 == 0 else "pB")
                        for ic in range(IC)
                    ]
                    pe_fills.append(lambda: norms(2))
                    pe_fills.append(lambda: norms(3))
                elif h == 2:
                    pe_fills = [lambda ib=ib: yout(0, ib, "pB" if ib % 2 == 0 else "pA") for ib in range(8)]
                else:
                    pe_fills = [lambda ib=ib: yout(0, ib, "pB" if ib % 2 == 0 else "pA") for ib in range(8, NB)]
                fills = iter(pe_fills)

                def fill(n=1):
                    for _ in range(n):
                        f = next(fills, None)
                        if f is not None:
                            f()

                # B-half normalize of previous head early in this phase
                ts0 = pend_st.pop((h, 0))
                sqrt_psum_direct(h, 0, ts0)
                if h > 0:
                    norm_half(h - 1, 1, pend_pv.pop(h - 1), "pA")
                for jb in (1, 2, 3):
                    ts = st_halves(h, jb)
                    sqrt_psum_direct(h, jb, ts)
                    fill(1)
                for g in (1, 2, 3):
                    for jb in range(4 * g, 4 * g + 4):
                        ts = st_halves(h, jb)
                        st_cast(h, jb, ts)
                        fill(1)
                    sqrt_group(g)
                fill(100)

                # ---------- exp phase ----------
                pvA = ps.tile([65, 1024], dt.float32, tag="pA", name="pvA")
                pvB = ps.tile([65, 1024], dt.float32, tag="pB", name="pvB")
                for c in range(NEC):
                    exp_chunk(h, c, pvA if c < 4 else pvB)
                    if c == 3:
                        raw_den(h, 0, pvA)
                        norm_half(h, 0, pvA, "pA")
                    if c == 5 and h == 3:
                        # pair-1 out-proj for the first i-half can start:
                        # heads 2,3 A-half normalized
                        for ib in range(4):
                            yout(1, ib, "pA")
                if h < HPC - 1:
                    pend_st[(h + 1, 0)] = st_halves(h + 1, 0)
                raw_den(h, 1, pvB)
                pend_pv[h] = pvB
                if h == 3:
                    for ib in range(4, 8):
                        yout(1, ib, "pA")

            # ---------- tail: B-half normalize of head 3, pair-1 y ----------
            norm_half(3, 1, pend_pv.pop(3), "pB")
            for ib in range(8, NB):
                yout(1, ib, "pB" if ib % 2 == 0 else "pA")

    nc.compile()
    return nc


def _prep_in_maps(x, wq, bq, wk, bk, wv, wo):
    f16 = np.float16
    in_maps = []
    xTs = [np.ascontiguousarray(x[b].T).astype(f16) for b in range(B)]
    for c in range(8):
        b, hg = divmod(c, HPC)
        hs = hg * HS
        biases = np.stack(
            [
                bq[hs : hs + 128],
                bq[hs + 128 : hs + 256],
                -2.0 * bk[hs : hs + 128],
                -2.0 * bk[hs + 128 : hs + 256],
            ],
            axis=1,
        ).astype(np.float32)
        in_maps.append(
            {
                "xT": xTs[b],
                "wq_t": np.ascontiguousarray(wq[hs : hs + HS, :].T).astype(f16),
                "wk_t": np.ascontiguousarray(-2.0 * wk[hs : hs + HS, :].T).astype(f16),
                "wv_t": np.ascontiguousarray(wv[hs : hs + HS, :].T).astype(f16),
                "woT": np.ascontiguousarray(wo[:, hs : hs + HS].T).astype(f16),
                "biases": np.ascontiguousarray(biases),
            }
        )
    return in_maps


def _get_nc():
    if "nc" not in _CACHE:
        _CACHE["nc"] = _build()
    return _CACHE["nc"]


def run(inputs, trace=False, **trace_kwargs):
    """Run on 8 cores; returns (full_output, BassKernelResults)."""
    from concourse.bass_utils import run_bass_kernel_spmd

    nc = _get_nc()
    wv_np = np.asarray(inputs["wv"], np.float32)
    bv_np = np.asarray(inputs["bv"], np.float32)
    wo_np = np.asarray(inputs["wo"], np.float32)
    in_maps = _prep_in_maps(
        np.asarray(inputs["x"], np.float32),
        np.asarray(inputs["wq"], np.float32), np.asarray(inputs["bq"], np.float32),
        np.asarray(inputs["wk"], np.float32), np.asarray(inputs["bk"], np.float32),
        wv_np, wo_np,
    )
    res = run_bass_kernel_spmd(nc, in_maps, list(range(8)), trace=trace, **trace_kwargs)
    # v-bias folds to wo @ bv after softmax normalization
    bo_eff = np.asarray(inputs["bo"], np.float32) + wo_np @ bv_np
    out = np.empty((B, N, D), np.float32)
    for b in range(B):
        acc = res.results[b * HPC]["y0"].astype(np.float32)
        acc = acc + res.results[b * HPC]["y1"]
        for c in range(b * HPC + 1, (b + 1) * HPC):
            acc = acc + res.results[c]["y0"]
            acc = acc + res.results[c]["y1"]
        out[b] = acc + bo_eff
    return out, res


def kernel(**inputs) -> np.ndarray:
    out, _ = run(inputs, trace=False)
    return out


if __name__ == "__main__":
    rng = np.random.default_rng(0)
    ins = {
        "x": rng.standard_normal((B, N, D)).astype(np.float32),
        "wq": (rng.standard_normal((D, D)) * 0.02).astype(np.float32),
        "bq": (rng.standard_normal(D) * 0.02).astype(np.float32),
        "wk": (rng.standard_normal((D, D)) * 0.02).astype(np.float32),
        "bk": (rng.standard_normal(D) * 0.02).astype(np.float32),
        "wv": (rng.standard_normal((D, D)) * 0.02).astype(np.float32),
        "bv": (rng.standard_normal(D) * 0.02).astype(np.float32),
        "wo": (rng.standard_normal((D, D)) * 0.02).astype(np.float32),
        "bo": (rng.standard_normal(D) * 0.02).astype(np.float32),
    }
    print(kernel(**ins).shape)


# revision 21
# speedup vs baseline: 1.1481x; 1.0447x over previous
"""L2-distance self-attention (B=2, N=2048, D=1024, H=16) on 8 trn2 NeuronCores.

Sharding: core c handles batch c//4 and heads 4*(c%4) .. 4*(c%4)+4.
Each core computes its 4 heads end-to-end and returns TWO (2048, 1024) fp16
partials of the output projection (head pair 0 and head pair 1); the host
sums the 8 partials per batch and adds bo_eff = bo + wo @ bv (the v-bias
contributes exactly wo@bv after softmax normalization, so it is folded out
of the device kernel).

Layout: q_aug rows = [qb(0-63); ones(64); q2(65)], k_stat rows =
[kb2(0-63); k2(64); ones(65)] so one K=66 matmul emits the full
d2[j,i] = q2[i] + k2[j] - 2 q.k.  Row 65 of q_aug is written by an
SBUF->SBUF DMA (engines cannot address single partitions above 64, DMA
can).  kb2 = -2*(x wk + bk) is host-prescaled via wk/bk.

Per-head pipeline (ACT is the bottleneck at ~64us/head):
  sqrt phase: j-blocks 0-7 are consumed by ACT Sqrt straight from PSUM
    (two [128,1024] half-ops per block); blocks 8-15 are drained by DVE
    copies into the s tile and ACT runs Sqrt in-place over two 4-block
    groups.  This splits the drain work between ACT and DVE so neither
    paces the other.
  exp phase: ACT Exp (scale=-1) over strided i-chunks; PE runs PV
    matmuls (v_aug ones column -> row 64 = softmax denominator) plus the
    first j-block of the NEXT head's d2 so the next sqrt phase starts hot.
  normalize: denominator -> fp32 -> reciprocal_approx_fast (from SBUF,
    not PSUM -- the custom DVE op misreads PSUM on HW), PE broadcast
    matmul, DVE multiply into oTp; done per i-half in DVE-idle exp phases
    (head 3's tail half in i-quarters to shorten the tail).
  out-proj: pair-0 y during heads 2/3 sqrt phases, pair-1 y overlapping
    exp(3) and the tail; separate DRAM tensor per pair.
"""

import sys

for p in ("/opt/trn_rl_repo", "/root/.axon_site/_ro/trn_rl_repo"):
    if p not in sys.path:
        sys.path.append(p)

import numpy as np

B, N, D, H = 2, 2048, 1024, 16
HD = 64          # head dim
HPC = 4          # heads per core
HS = HPC * HD    # head-group width per core (256)
NB = N // 128    # 16 j-blocks
IC = N // 512    # 4 i-chunks of 512
KB = D // 128    # 8 contraction blocks for projections
EC = 256         # exp/PV i-chunk width
NEC = N // EC    # 8 exp chunks per head
NDIR = 8         # j-blocks consumed psum-direct by ACT (rest DVE-drained)

_CACHE = {}


def _build():
    import concourse.bacc as bacc
    import concourse.mybir as mybir
    import concourse.tile as tile

    dt = mybir.dt
    AF = mybir.ActivationFunctionType
    ALU = mybir.AluOpType

    nc = bacc.Bacc("TRN2", target_bir_lowering=False, debug=False)

    # ---- DRAM I/O (per core) ----
    xT = nc.dram_tensor("xT", [D, N], dt.float16, kind="ExternalInput")
    wq = nc.dram_tensor("wq_t", [D, HS], dt.float16, kind="ExternalInput")
    wk = nc.dram_tensor("wk_t", [D, HS], dt.float16, kind="ExternalInput")
    wv = nc.dram_tensor("wv_t", [D, HS], dt.float16, kind="ExternalInput")
    wo = nc.dram_tensor("woT", [HS, D], dt.float16, kind="ExternalInput")
    bias_d = nc.dram_tensor("biases", [128, 4], dt.float32, kind="ExternalInput")
    y0 = nc.dram_tensor("y0", [N, D], dt.float16, kind="ExternalOutput")
    y1 = nc.dram_tensor("y1", [N, D], dt.float16, kind="ExternalOutput")
    ydram = [y0, y1]

    with tile.TileContext(nc) as tc:
        with (
            tc.tile_pool(name="cst", bufs=1) as cst,
            tc.tile_pool(name="u4", bufs=9) as u4,        # 4KB slots: xt, sq, yac
            tc.tile_pool(name="wp", bufs=1) as wp,
            tc.tile_pool(name="wop", bufs=1) as wop,
            tc.tile_pool(name="aug", bufs=1) as aug,
            tc.tile_pool(name="rawp", bufs=2) as rawp,    # raws[h] rotate
            tc.tile_pool(name="dp", bufs=1) as dpool,
            tc.tile_pool(name="spool", bufs=1) as spool,
            tc.tile_pool(name="e8", bufs=2) as e8,
            tc.tile_pool(name="psum", bufs=2, space="PSUM") as ps,
        ):
            # ---- constants ----
            ones_row = cst.tile([1, 512], dt.float16, tag="ones_row")
            nc.gpsimd.memset(ones_row[:], 1.0)
            ones64f = cst.tile([1, 64], dt.float32, tag="ones64f")
            nc.gpsimd.memset(ones64f[:], 1.0)
            # norm reduce matrix: col0 = 1 on rows 0-63 (q2 = sum qb^2),
            # col32 = 0.25 on rows 64-127 (k2 = 0.25*sum kb2^2)
            emat = cst.tile([128, 33], dt.float16, tag="emat")
            nc.gpsimd.memset(emat[:], 0.0)
            nc.gpsimd.memset(emat[0:64, 0:1], 1.0)
            nc.gpsimd.memset(emat[64:128, 32:33], 0.25)

            bias_pp = cst.tile([128, 4], dt.float32, tag="bias_pp")
            nc.sync.dma_start(bias_pp[:], bias_d[:, :])

            # ---- per-head tiles ----
            q_aug = [aug.tile([66, N], dt.float16, tag=f"qa{h}", name=f"qa{h}") for h in range(HPC)]
            k_stat = [aug.tile([66, N], dt.float16, tag=f"ks{h}", name=f"ks{h}") for h in range(HPC)]
            for h in range(HPC):
                # q_aug row 64 = ones (const); row 65 overwritten with q2 by
                # DMA.  k_stat row 65 = ones (const); row 64 overwritten
                # with k2 by a DVE copy (base-64 is engine-addressable).
                nc.gpsimd.memset(q_aug[h][64:66, :], 1.0)
                nc.gpsimd.memset(k_stat[h][64:66, :], 1.0)
            v_jb = [aug.tile([128, HPC * 65], dt.float16, tag=f"v{jb}", name=f"v{jb}") for jb in range(NB)]
            for jb in range(NB):
                nc.gpsimd.memset(
                    v_jb[jb][:].rearrange("p (b d) -> p b d", d=65)[:, :, 64:65], 1.0
                )
            oTp = [
                aug.tile([128, N], dt.float16, tag="oTp0", name="oTp0"),
                aug.tile([128, N], dt.float16, tag="oTp1", name="oTp1"),
            ]

            # PE warmup: dependency-free matmuls release the HAM clock gate
            wup = ps.tile([128, 512], dt.float32, tag="pA", name="wup")
            for r in range(12):
                nc.tensor.matmul(
                    wup[:], ones_row[0:1, 0:128], ones_row[0:1, :],
                    start=(r == 0), stop=(r == 11),
                )

            # ---- input DMA ----
            xt = [u4.tile([128, N], dt.float16, tag="u4", name=f"xt{k}") for k in range(KB)]
            wq_all = wp.tile([128, KB * HS], dt.float16, tag="wq_all")
            wk_all = wp.tile([128, KB * HS], dt.float16, tag="wk_all")
            wv_all = wp.tile([128, KB * HS], dt.float16, tag="wv_all")
            for k in range(KB):
                nc.sync.dma_start(
                    wq_all[:, k * HS : (k + 1) * HS], wq[k * 128 : (k + 1) * 128, :]
                )
                nc.sync.dma_start(xt[k][:], xT[k * 128 : (k + 1) * 128, :])
            for k in range(KB):
                nc.sync.dma_start(
                    wk_all[:, k * HS : (k + 1) * HS], wk[k * 128 : (k + 1) * 128, :]
                )
            for k in range(KB):
                nc.gpsimd.dma_start(
                    wv_all[:, k * HS : (k + 1) * HS], wv[k * 128 : (k + 1) * 128, :]
                )
            wotp = [wop.tile([128, D], dt.float16, tag=f"wop{p}", name=f"wop{p}") for p in range(2)]
            for p in range(2):
                nc.gpsimd.dma_start(wotp[p][:], wo[p * 128 : (p + 1) * 128, :])

            # ---- big SBUF tiles ----
            s = spool.tile([128, NB * N], dt.float16, tag="s")
            sv = s[:].rearrange("p (t i) -> p t i", t=NB)

            raws = [None] * HPC
            pend_st = {}   # (h, jb) -> (tileA, tileB) kept in PSUM
            pend_pv = {}   # h -> pvB psum tile (den row consumed next phase)

            ic_sl = lambda ic: slice(ic * 512, (ic + 1) * 512)

            # ---- helpers (emission) ----
            def proj_qk_ic(w_all, bcol, dest, m, ic, tag):
                # heads 2m, 2m+1; psum (128 d, 512 i); bias fused in copy
                p = ps.tile([128, 512], dt.float32, tag=tag, name="pp")
                for k in range(KB):
                    nc.tensor.matmul(
                        p[:],
                        w_all[:, k * HS + m * 128 : k * HS + (m + 1) * 128],
                        xt[k][:, ic_sl(ic)],
                        start=(k == 0), stop=(k == KB - 1),
                    )
                for half in range(2):
                    with nc.allow_low_precision(reason="fp16 activations"):
                        nc.vector.tensor_scalar_add(
                            out=dest[2 * m + half][0:64, ic_sl(ic)],
                            in0=p[64 * half : 64 * half + 64, :],
                            scalar1=bias_pp[64 * half : 64 * half + 64, bcol + m : bcol + m + 1],
                        )

            def sq_half(h, which, sq, half):
                # squares for one i-half (1024 cols)
                lo = half * 1024
                src = q_aug[h] if which == "q" else k_stat[h]
                r0 = 0 if which == "q" else 64
                nc.vector.tensor_tensor(
                    out=sq[r0 : r0 + 64, lo : lo + 1024],
                    in0=src[0:64, lo : lo + 1024], in1=src[0:64, lo : lo + 1024],
                    op=ALU.mult,
                )

            def norm_red(h, which, sq, half, tag):
                # q2 (row 0) and k2 (row 32) of psum; q2 -> stage -> DMA to
                # q_aug row 65, k2 -> k_stat row 64 (direct DVE copy)
                lo = half * 1024
                p = ps.tile([33, 1024], dt.float32, tag=tag, name="np")
                for d_ in range(2):
                    ic = 2 * half + d_
                    nc.tensor.matmul(
                        p[:, d_ * 512 : (d_ + 1) * 512], emat[:], sq[:, ic_sl(ic)],
                        start=True, stop=True,
                    )
                with nc.allow_low_precision(reason="fp16 stats"):
                    if which == "q":
                        st_ = dpool.tile([1, 1024], dt.float16, tag="q2st", bufs=2, name="q2st")
                        nc.vector.tensor_copy(st_[:], p[0:1, :])
                        nc.gpsimd.dma_start(q_aug[h][65:66, lo : lo + 1024], st_[:])
                    else:
                        nc.vector.tensor_copy(k_stat[h][64:65, lo : lo + 1024], p[32:33, :])

            def vp_one(jb):
                p = ps.tile([128, HS], dt.float32, tag="pA" if jb % 2 == 0 else "pB", name="vp")
                for k in range(KB):
                    nc.tensor.matmul(
                        p[:], xt[k][:, jb * 128 : (jb + 1) * 128],
                        wv_all[:, k * HS : (k + 1) * HS],
                        start=(k == 0), stop=(k == KB - 1),
                    )
                dst = v_jb[jb][:].rearrange("p (h d) -> p h d", d=65)[:, :, 0:64]
                nc.vector.tensor_copy(dst, p[:].rearrange("p (h d) -> p h d", d=64))

            def st_halves(h, jb):
                # full d2 for j-block jb, both i-halves, kept in PSUM
                ts = []
                for half, tag in ((0, "pA"), (1, "pB")):
                    t_ = ps.tile([128, 1024], dt.float32, tag=tag, name=f"st{half}")
                    for d_ in range(2):
                        ic = 2 * half + d_
                        nc.tensor.matmul(
                            t_[:, d_ * 512 : (d_ + 1) * 512],
                            k_stat[h][0:66, jb * 128 : (jb + 1) * 128],
                            q_aug[h][0:66, ic_sl(ic)],
                            start=True, stop=True,
                        )
                    ts.append(t_)
                return ts

            def st_cast(h, jb, ts):
                # drain d2 psum -> s (fp16); sqrt later runs in-place
                for half in (0, 1):
                    with nc.allow_low_precision(reason="fp16 d2"):
                        nc.vector.tensor_copy(
                            s[:, jb * N + half * 1024 : jb * N + (half + 1) * 1024],
                            ts[half][:],
                        )

            def sqrt_psum_direct(h, jb, ts):
                for half in (0, 1):
                    nc.scalar.activation(
                        s[:, jb * N + half * 1024 : jb * N + (half + 1) * 1024],
                        ts[half][:], AF.Sqrt,
                    )

            def sqrt_group(g):
                lo, hi = 4 * g * N, 4 * (g + 1) * N
                nc.scalar.activation(s[:, lo:hi], s[:, lo:hi], AF.Sqrt)

            def exp_chunk(h, c, pvh):
                e = e8.tile([128, NB * EC], dt.float16, tag="e8", name="e")
                nc.scalar.activation(
                    e[:].rearrange("p (t i) -> p t i", t=NB),
                    sv[:, :, c * EC : (c + 1) * EC],
                    AF.Exp, scale=-1.0,
                )
                cc = (c % 4) * EC
                for t in range(NB):
                    nc.tensor.matmul(
                        pvh[:, cc : cc + EC],
                        v_jb[t][:, h * 65 : h * 65 + 65],
                        e[:, t * EC : (t + 1) * EC],
                        start=(t == 0), stop=(t == NB - 1),
                    )

            def raw_part(h, lo, w, pvh, plo):
                # pv rows 0-63 -> raws fp16 (row 64 = den stays in psum)
                if lo == 0:
                    raws[h] = rawp.tile([64, N], dt.float16, tag="raw", name=f"raw{h}")
                with nc.allow_low_precision(reason="fp16 softmax weights"):
                    nc.vector.tensor_copy(raws[h][:, lo : lo + w], pvh[0:64, plo : plo + w])

            def norm_part(h, lo, w, pvh, plo, tag):
                # den (psum) -> sbuf fp32 -> approx reciprocal -> PE
                # broadcast -> multiply raws into oTp
                den = dpool.tile([1, 1024], dt.float32, tag="den", bufs=1, name="den")
                nc.vector.tensor_copy(den[0:1, 0:w], pvh[64:65, plo : plo + w])
                dinv = dpool.tile([1, 1024], dt.float32, tag="dinv", bufs=1, name="dinv")
                nc.vector.reciprocal_approx_fast(out=dinv[0:1, 0:w], in_=den[0:1, 0:w])
                bc = ps.tile([64, 1024], dt.float32, tag=tag, name="bc")
                for d_ in range(w // 512):
                    nc.tensor.matmul(
                        bc[:, d_ * 512 : (d_ + 1) * 512],
                        ones64f[:],
                        dinv[0:1, d_ * 512 : (d_ + 1) * 512],
                        start=True, stop=True,
                    )
                row = 64 * (h % 2)
                with nc.allow_low_precision(reason="fp16 softmax weights"):
                    nc.vector.tensor_tensor(
                        out=oTp[h // 2][row : row + 64, lo : lo + w],
                        in0=raws[h][:, lo : lo + w], in1=bc[:, 0:w], op=ALU.mult,
                    )

            def norms_head(h):
                sq = u4.tile([128, N], dt.float16, tag="u4", name=f"sqh{h}")
                for half in range(2):
                    sq_half(h, "q", sq, half)
                    sq_half(h, "k", sq, half)
                    norm_red(h, "q", sq, half, "pA")
                    norm_red(h, "k", sq, half, "pB")

            def yout(pair, ib, tag, act_copy=False):
                yp = ps.tile([128, D], dt.float32, tag=tag, name="yp")
                for fc in range(2):
                    nc.tensor.matmul(
                        yp[:, fc * 512 : (fc + 1) * 512],
                        oTp[pair][:, ib * 128 : (ib + 1) * 128],
                        wotp[pair][:, fc * 512 : (fc + 1) * 512],
                        start=True, stop=True,
                    )
                yac = u4.tile([128, D], dt.float16, tag="u4", name="yac")
                with nc.allow_low_precision(reason="fp16 partial output"):
                    if act_copy:
                        nc.scalar.copy(yac[:], yp[:])
                    else:
                        nc.vector.tensor_copy(yac[:], yp[:])
                nc.sync.dma_start(ydram[pair][ib * 128 : (ib + 1) * 128, :], yac[:])

            # ================= emission schedule =================
            # ---- lead-in: pair-0 projections + stats, first d2 blocks ----
            sq0 = u4.tile([128, N], dt.float16, tag="u4", name="sq0")
            # rows 64-127 are read (x0) by the q norm_reds before the k
            # squares land -- uninitialized NaN bits would poison the PE sum
            nc.gpsimd.memset(sq0[64:128, :], 0.0)
            for ic in range(2):
                proj_qk_ic(wq_all, 0, q_aug, 0, ic, "pA" if ic % 2 == 0 else "pB")
            sq_half(0, "q", sq0, 0)
            for ic in range(2, IC):
                proj_qk_ic(wq_all, 0, q_aug, 0, ic, "pA" if ic % 2 == 0 else "pB")
            sq_half(0, "q", sq0, 1)
            norm_red(0, "q", sq0, 0, "pA")
            norm_red(0, "q", sq0, 1, "pB")
            for ic in range(2):
                proj_qk_ic(wk_all, 2, k_stat, 0, ic, "pA" if ic % 2 == 0 else "pB")
            sq_half(0, "k", sq0, 0)
            norm_red(0, "k", sq0, 0, "pA")
            for ic in range(2, IC):
                proj_qk_ic(wk_all, 2, k_stat, 0, ic, "pA" if ic % 2 == 0 else "pB")
            sq_half(0, "k", sq0, 1)
            norm_red(0, "k", sq0, 1, "pB")
            pend_st[(0, 0)] = st_halves(0, 0)

            # ---- per-head phases ----
            for h in range(HPC):
                # ---------- sqrt phase ----------
                if h == 0:
                    pe_fills = [lambda jb=jb: vp_one(jb) for jb in range(NB)]
                    pe_fills.append(lambda: norms_head(1))
                elif h == 1:
                    pe_fills = [
                        lambda ic=ic: proj_qk_ic(wq_all, 0, q_aug, 1, ic, "pA" if ic % 2 == 0 else "pB")
                        for ic in range(IC)
                    ]
                    pe_fills += [
                        lambda ic=ic: proj_qk_ic(wk_all, 2, k_stat, 1, ic, "pA" if ic % 2 == 0 else "pB")
                        for ic in range(IC)
                    ]
                    pe_fills.append(lambda: norms_head(2))
                    pe_fills.append(lambda: norms_head(3))
                elif h == 2:
                    pe_fills = [lambda ib=ib: yout(0, ib, "pB" if ib % 2 == 0 else "pA") for ib in range(8)]
                else:
                    pe_fills = [lambda ib=ib: yout(0, ib, "pB" if ib % 2 == 0 else "pA") for ib in range(8, NB)]
                fills = iter(pe_fills)

                def fill(n=1):
                    for _ in range(n):
                        f = next(fills, None)
                        if f is not None:
                            f()

                # B-half normalize of previous head early in this phase
                ts0 = pend_st.pop((h, 0))
                sqrt_psum_direct(h, 0, ts0)
                if h > 0:
                    pvb = pend_pv.pop(h - 1)
                    raw_part(h - 1, 1024, 1024, pvb, 0)
                    norm_part(h - 1, 1024, 1024, pvb, 0, "pA")
                for jb in range(1, NDIR):
                    ts = st_halves(h, jb)
                    sqrt_psum_direct(h, jb, ts)
                    fill(1)
                for g in (NDIR // 4, NDIR // 4 + 1):
                    for jb in range(4 * g, 4 * g + 4):
                        ts = st_halves(h, jb)
                        st_cast(h, jb, ts)
                        fill(1)
                    sqrt_group(g)
                fill(100)

                # ---------- exp phase ----------
                pvA = ps.tile([65, 1024], dt.float32, tag="pA", name="pvA")
                pvB = ps.tile([65, 1024], dt.float32, tag="pB", name="pvB")
                for c in range(NEC):
                    exp_chunk(h, c, pvA if c < 4 else pvB)
                    if c == 3:
                        raw_part(h, 0, 1024, pvA, 0)
                        norm_part(h, 0, 1024, pvA, 0, "pA")
                    if c == 5 and h == 3:
                        # head-3 B first quarter (cols 1024-1535) is final
                        raw_part(3, 1024, 512, pvB, 0)
                        norm_part(3, 1024, 512, pvB, 0, "pA")
                        for ib in range(8, 12):
                            yout(1, ib, "pA")
                if h < HPC - 1:
                    pend_st[(h + 1, 0)] = st_halves(h + 1, 0)
                    raw_part(h, 1024, 1024, pvB, 0)
                    pend_pv[h] = pvB
                if h == 3:
                    for ib in range(4):
                        yout(1, ib, "pA")

            # ---------- tail: head-3 B second quarter, rest of pair-1 y ----
            raw_part(3, 1536, 512, pvB, 512)
            norm_part(3, 1536, 512, pvB, 512, "pB")
            for i_, ib in enumerate(list(range(12, NB)) + list(range(4, 8))):
                yout(1, ib, "pB" if ib % 2 == 0 else "pA", act_copy=(i_ % 2 == 0))

    nc.compile()
    return nc


def _prep_in_maps(x, wq, bq, wk, bk, wv, wo):
    f16 = np.float16
    in_maps = []
    xTs = [np.ascontiguousarray(x[b].T).astype(f16) for b in range(B)]
    for c in range(8):
        b, hg = divmod(c, HPC)
        hs = hg * HS
        biases = np.stack(
            [
                bq[hs : hs + 128],
                bq[hs + 128 : hs + 256],
                -2.0 * bk[hs : hs + 128],
                -2.0 * bk[hs + 128 : hs + 256],
            ],
            axis=1,
        ).astype(np.float32)
        in_maps.append(
            {
                "xT": xTs[b],
                "wq_t": np.ascontiguousarray(wq[hs : hs + HS, :].T).astype(f16),
                "wk_t": np.ascontiguousarray(-2.0 * wk[hs : hs + HS, :].T).astype(f16),
                "wv_t": np.ascontiguousarray(wv[hs : hs + HS, :].T).astype(f16),
                "woT": np.ascontiguousarray(wo[:, hs : hs + HS].T).astype(f16),
                "biases": np.ascontiguousarray(biases),
            }
        )
    return in_maps


def _get_nc():
    if "nc" not in _CACHE:
        _CACHE["nc"] = _build()
    return _CACHE["nc"]


def run(inputs, trace=False, **trace_kwargs):
    """Run on 8 cores; returns (full_output, BassKernelResults)."""
    from concourse.bass_utils import run_bass_kernel_spmd

    nc = _get_nc()
    wv_np = np.asarray(inputs["wv"], np.float32)
    bv_np = np.asarray(inputs["bv"], np.float32)
    wo_np = np.asarray(inputs["wo"], np.float32)
    in_maps = _prep_in_maps(
        np.asarray(inputs["x"], np.float32),
        np.asarray(inputs["wq"], np.float32), np.asarray(inputs["bq"], np.float32),
        np.asarray(inputs["wk"], np.float32), np.asarray(inputs["bk"], np.float32),
        wv_np, wo_np,
    )
    res = run_bass_kernel_spmd(nc, in_maps, list(range(8)), trace=trace, **trace_kwargs)
    # v-bias folds to wo @ bv after softmax normalization
    bo_eff = np.asarray(inputs["bo"], np.float32) + wo_np @ bv_np
    out = np.empty((B, N, D), np.float32)
    for b in range(B):
        acc = res.results[b * HPC]["y0"].astype(np.float32)
        acc = acc + res.results[b * HPC]["y1"].astype(np.float32)
        for c in range(b * HPC + 1, (b + 1) * HPC):
            acc = acc + res.results[c]["y0"].astype(np.float32)
            acc = acc + res.results[c]["y1"].astype(np.float32)
        out[b] = acc + bo_eff
    return out, res


def kernel(**inputs) -> np.ndarray:
    out, _ = run(inputs, trace=False)
    return out


if __name__ == "__main__":
    rng = np.random.default_rng(0)
    ins = {
        "x": rng.standard_normal((B, N, D)).astype(np.float32),
        "wq": (rng.standard_normal((D, D)) * 0.02).astype(np.float32),
        "bq": (rng.standard_normal(D) * 0.02).astype(np.float32),
        "wk": (rng.standard_normal((D, D)) * 0.02).astype(np.float32),
        "bk": (rng.standard_normal(D) * 0.02).astype(np.float32),
        "wv": (rng.standard_normal((D, D)) * 0.02).astype(np.float32),
        "bv": (rng.standard_normal(D) * 0.02).astype(np.float32),
        "wo": (rng.standard_normal((D, D)) * 0.02).astype(np.float32),
        "bo": (rng.standard_normal(D) * 0.02).astype(np.float32),
    }
    print(kernel(**ins).shape)


# revision 33
# speedup vs baseline: 1.1797x; 1.0276x over previous
"""L2-distance self-attention (B=2, N=2048, D=1024, H=16) on 8 trn2 NeuronCores.

Sharding: core c handles batch c//4 and heads 4*(c%4) .. 4*(c%4)+4.
Each core computes its 4 heads end-to-end and returns TWO (2048, 1024) fp16
partials of the output projection (head pair 0 and head pair 1); the host
sums the 8 partials per batch and adds bo_eff = bo + wo @ bv (the v-bias
contributes exactly wo@bv after softmax normalization, so it is folded out
of the device kernel).

Layout: q_aug rows = [qb(0-63); ones(64); q2(65)], k_stat rows =
[kb2(0-63); k2(64); ones(65)] so one K=66 matmul emits the full
d2[j,i] = q2[i] + k2[j] - 2 q.k.  Row 65 of q_aug is written by an
SBUF->SBUF DMA (engines cannot address single partitions above 64, DMA
can).  kb2 = -2*(x wk + bk) is host-prescaled via wk/bk.

Per-head pipeline (ACT is the bottleneck at ~64us/head):
  sqrt phase: j-blocks 0-7 are consumed by ACT Sqrt straight from PSUM
    (two [128,1024] half-ops per block); blocks 8-15 are drained by DVE
    copies into the s tile and ACT runs Sqrt in-place over two 4-block
    groups.  This splits the drain work between ACT and DVE so neither
    paces the other.
  exp phase: ACT Exp (scale=-1) over strided i-chunks; PE runs PV
    matmuls (v_aug ones column -> row 64 = softmax denominator) plus the
    first j-block of the NEXT head's d2 so the next sqrt phase starts hot.
  normalize: denominator -> fp32 -> reciprocal_approx_fast (from SBUF,
    not PSUM -- the custom DVE op misreads PSUM on HW), PE broadcast
    matmul, DVE multiply into oTp; done per i-half in DVE-idle exp phases
    (head 3's tail half in i-quarters to shorten the tail).
  out-proj: pair-0 y during heads 2/3 sqrt phases, pair-1 y overlapping
    exp(3) and the tail; separate DRAM tensor per pair.
"""

import sys

for p in ("/opt/trn_rl_repo", "/root/.axon_site/_ro/trn_rl_repo"):
    if p not in sys.path:
        sys.path.append(p)

import numpy as np

B, N, D, H = 2, 2048, 1024, 16
HD = 64          # head dim
HPC = 4          # heads per core
HS = HPC * HD    # head-group width per core (256)
NB = N // 128    # 16 j-blocks
IC = N // 512    # 4 i-chunks of 512
KB = D // 128    # 8 contraction blocks for projections
EC = 256         # exp/PV i-chunk width
NEC = N // EC    # 8 exp chunks per head
NDIR = 8         # j-blocks consumed psum-direct by ACT (rest DVE-drained)

_CACHE = {}


def _build():
    import concourse.bacc as bacc
    import concourse.mybir as mybir
    import concourse.tile as tile

    dt = mybir.dt
    AF = mybir.ActivationFunctionType
    ALU = mybir.AluOpType

    nc = bacc.Bacc("TRN2", target_bir_lowering=False, debug=False)

    # ---- DRAM I/O (per core) ----
    xT = nc.dram_tensor("xT", [D, N], dt.float16, kind="ExternalInput")
    wq = nc.dram_tensor("wq_t", [D, HS], dt.float16, kind="ExternalInput")
    wk = nc.dram_tensor("wk_t", [D, HS], dt.float16, kind="ExternalInput")
    wv = nc.dram_tensor("wv_t", [D, HS], dt.float16, kind="ExternalInput")
    wo = nc.dram_tensor("woT", [HS, D], dt.float16, kind="ExternalInput")
    bias_d = nc.dram_tensor("biases", [128, 4], dt.float32, kind="ExternalInput")
    y0 = nc.dram_tensor("y0", [N, D], dt.float16, kind="ExternalOutput")
    y1 = nc.dram_tensor("y1", [N, D], dt.float16, kind="ExternalOutput")
    ydram = [y0, y1]

    with tile.TileContext(nc) as tc:
        with (
            tc.tile_pool(name="cst", bufs=1) as cst,
            tc.tile_pool(name="u4", bufs=9) as u4,        # 4KB slots: xt, sq, yac
            tc.tile_pool(name="wp", bufs=1) as wp,
            tc.tile_pool(name="wop", bufs=1) as wop,
            tc.tile_pool(name="aug", bufs=1) as aug,
            tc.tile_pool(name="rawp", bufs=2) as rawp,    # raws[h] rotate
            tc.tile_pool(name="dp", bufs=1) as dpool,
            tc.tile_pool(name="spool", bufs=1) as spool,
            tc.tile_pool(name="e8", bufs=2) as e8,
            tc.tile_pool(name="psum", bufs=2, space="PSUM") as ps,
        ):
            # ---- constants ----
            ones_row = cst.tile([1, 512], dt.float16, tag="ones_row")
            nc.gpsimd.memset(ones_row[:], 1.0)
            ones64f = cst.tile([1, 64], dt.float32, tag="ones64f")
            nc.gpsimd.memset(ones64f[:], 1.0)
            # norm reduce matrix: col0 = 1 on rows 0-63 (q2 = sum qb^2),
            # col32 = 0.25 on rows 64-127 (k2 = 0.25*sum kb2^2)
            emat = cst.tile([128, 33], dt.float16, tag="emat")
            nc.gpsimd.memset(emat[:], 0.0)
            nc.gpsimd.memset(emat[0:64, 0:1], 1.0)
            nc.gpsimd.memset(emat[64:128, 32:33], 0.25)

            bias_pp = cst.tile([128, 4], dt.float32, tag="bias_pp")
            nc.sync.dma_start(bias_pp[:], bias_d[:, :])
            # exp shift: e' = exp(5 - s); cancels in the softmax normalize
            bias5 = cst.tile([128, 1], dt.float32, tag="bias5")
            nc.gpsimd.memset(bias5[:], 5.0)

            # ---- per-head tiles ----
            q_aug = [aug.tile([66, N], dt.float16, tag=f"qa{h}", name=f"qa{h}") for h in range(HPC)]
            k_stat = [aug.tile([66, N], dt.float16, tag=f"ks{h}", name=f"ks{h}") for h in range(HPC)]
            for h in range(HPC):
                # q_aug row 64 = ones (const); row 65 overwritten with q2 by
                # DMA.  k_stat row 65 = ones (const); row 64 overwritten
                # with k2 by a DVE copy (base-64 is engine-addressable).
                nc.gpsimd.memset(q_aug[h][64:66, :], 1.0)
                nc.gpsimd.memset(k_stat[h][64:66, :], 1.0)
            # v per j-block: [p, h(4), d(65)]; d 64 = ones column -> PV row
            # 64 = softmax denominator
            v_jb = [aug.tile([128, HPC * 65], dt.float16, tag=f"v{jb}", name=f"v{jb}") for jb in range(NB)]
            for jb in range(NB):
                nc.gpsimd.memset(
                    v_jb[jb][:].rearrange("p (b d) -> p b d", d=65)[:, :, 64:65], 1.0
                )
            oTp = [
                aug.tile([128, N], dt.float16, tag="oTp0", name="oTp0"),
                aug.tile([128, N], dt.float16, tag="oTp1", name="oTp1"),
            ]

            # PE warmup: dependency-free matmuls release the HAM clock gate
            wup = ps.tile([128, 512], dt.float32, tag="pA", name="wup")
            for r in range(12):
                nc.tensor.matmul(
                    wup[:], ones_row[0:1, 0:128], ones_row[0:1, :],
                    start=(r == 0), stop=(r == 11),
                )

            # ---- input DMA ----
            xt = [u4.tile([128, N], dt.float16, tag="u4", name=f"xt{k}") for k in range(KB)]
            wq_all = wp.tile([128, KB * HS], dt.float16, tag="wq_all")
            wk_all = wp.tile([128, KB * HS], dt.float16, tag="wk_all")
            wv_all = wp.tile([128, KB * HS], dt.float16, tag="wv_all")
            for k in range(KB):
                nc.sync.dma_start(
                    wq_all[:, k * HS : (k + 1) * HS], wq[k * 128 : (k + 1) * 128, :]
                )
                nc.sync.dma_start(xt[k][:], xT[k * 128 : (k + 1) * 128, :])
            for k in range(KB):
                nc.sync.dma_start(
                    wk_all[:, k * HS : (k + 1) * HS], wk[k * 128 : (k + 1) * 128, :]
                )
            wotp = [wop.tile([128, D], dt.float16, tag=f"wop{p}", name=f"wop{p}") for p in range(2)]

            # ---- big SBUF tiles ----
            s = spool.tile([128, NB * N], dt.float16, tag="s")
            sv = s[:].rearrange("p (t i) -> p t i", t=NB)

            raws = [None] * HPC
            pend_st = {}   # (h, jb) -> (tileA, tileB) kept in PSUM
            pend_pv = {}   # h -> pvB psum tile (den row consumed next phase)

            ic_sl = lambda ic: slice(ic * 512, (ic + 1) * 512)

            # ---- helpers (emission) ----
            def proj_qk_ic(w_all, bcol, dest, m, ic, tag):
                # heads 2m, 2m+1; psum (128 d, 512 i); bias fused in copy
                p = ps.tile([128, 512], dt.float32, tag=tag, name="pp")
                for k in range(KB):
                    nc.tensor.matmul(
                        p[:],
                        w_all[:, k * HS + m * 128 : k * HS + (m + 1) * 128],
                        xt[k][:, ic_sl(ic)],
                        start=(k == 0), stop=(k == KB - 1),
                    )
                for half in range(2):
                    with nc.allow_low_precision(reason="fp16 activations"):
                        nc.vector.tensor_scalar_add(
                            out=dest[2 * m + half][0:64, ic_sl(ic)],
                            in0=p[64 * half : 64 * half + 64, :],
                            scalar1=bias_pp[64 * half : 64 * half + 64, bcol + m : bcol + m + 1],
                        )

            def sq_half(h, which, sq, half):
                # squares for one i-half (1024 cols)
                lo = half * 1024
                src = q_aug[h] if which == "q" else k_stat[h]
                r0 = 0 if which == "q" else 64
                nc.vector.tensor_tensor(
                    out=sq[r0 : r0 + 64, lo : lo + 1024],
                    in0=src[0:64, lo : lo + 1024], in1=src[0:64, lo : lo + 1024],
                    op=ALU.mult,
                )

            def norm_red(h, which, sq, half, tag):
                # q2 (row 0) and k2 (row 32) of psum; q2 -> stage -> DMA to
                # q_aug row 65, k2 -> k_stat row 64 (direct DVE copy)
                lo = half * 1024
                p = ps.tile([33, 1024], dt.float32, tag=tag, name="np")
                for d_ in range(2):
                    ic = 2 * half + d_
                    nc.tensor.matmul(
                        p[:, d_ * 512 : (d_ + 1) * 512], emat[:], sq[:, ic_sl(ic)],
                        start=True, stop=True,
                    )
                with nc.allow_low_precision(reason="fp16 stats"):
                    if which == "q":
                        st_ = dpool.tile([1, 1024], dt.float16, tag="q2st", bufs=2, name="q2st")
                        nc.vector.tensor_copy(st_[:], p[0:1, :])
                        nc.gpsimd.dma_start(q_aug[h][65:66, lo : lo + 1024], st_[:])
                    else:
                        nc.vector.tensor_copy(k_stat[h][64:65, lo : lo + 1024], p[32:33, :])

            def vp_one(jb):
                p = ps.tile([128, HS], dt.float32, tag="pA" if jb % 2 == 0 else "pB", name="vp")
                for k in range(KB):
                    nc.tensor.matmul(
                        p[:], xt[k][:, jb * 128 : (jb + 1) * 128],
                        wv_all[:, k * HS : (k + 1) * HS],
                        start=(k == 0), stop=(k == KB - 1),
                    )
                dst = v_jb[jb][:].rearrange("p (h d) -> p h d", d=65)[:, :, 0:64]
                nc.vector.tensor_copy(dst, p[:].rearrange("p (h d) -> p h d", d=64))

            def st_halves(h, jb):
                # full d2 for j-block jb, both i-halves, kept in PSUM
                ts = []
                for half, tag in ((0, "pA"), (1, "pB")):
                    t_ = ps.tile([128, 1024], dt.float32, tag=tag, name=f"st{half}")
                    for d_ in range(2):
                        ic = 2 * half + d_
                        nc.tensor.matmul(
                            t_[:, d_ * 512 : (d_ + 1) * 512],
                            k_stat[h][0:66, jb * 128 : (jb + 1) * 128],
                            q_aug[h][0:66, ic_sl(ic)],
                            start=True, stop=True,
                        )
                    ts.append(t_)
                return ts

            def st_cast(h, jb, ts):
                # drain d2 psum -> s (fp16); sqrt later runs in-place
                for half in (0, 1):
                    with nc.allow_low_precision(reason="fp16 d2"):
                        nc.vector.tensor_copy(
                            s[:, jb * N + half * 1024 : jb * N + (half + 1) * 1024],
                            ts[half][:],
                        )

            def sqrt_psum_direct(h, jb, ts):
                for half in (0, 1):
                    nc.scalar.activation(
                        s[:, jb * N + half * 1024 : jb * N + (half + 1) * 1024],
                        ts[half][:], AF.Sqrt,
                    )

            def sqrt_group(g):
                lo, hi = 4 * g * N, 4 * (g + 1) * N
                nc.scalar.activation(s[:, lo:hi], s[:, lo:hi], AF.Sqrt)

            def exp_chunk(h, c, pvh):
                e = e8.tile([128, NB * EC], dt.float16, tag="e8", name="e")
                nc.scalar.activation(
                    e[:].rearrange("p (t i) -> p t i", t=NB),
                    sv[:, :, c * EC : (c + 1) * EC],
                    AF.Exp, scale=-1.0,
                )
                cc = (c % 4) * EC
                for t in range(NB):
                    nc.tensor.matmul(
                        pvh[:, cc : cc + EC],
                        v_jb[t][:, h * 65 : h * 65 + 65],
                        e[:, t * EC : (t + 1) * EC],
                        start=(t == 0), stop=(t == NB - 1),
                    )

            def raw_part(h, lo, w, pvh, plo):
                # pv rows 0-63 -> raws fp16 (row 64 = den stays in psum)
                if lo == 0:
                    raws[h] = rawp.tile([64, N], dt.float16, tag="raw", name=f"raw{h}")
                with nc.allow_low_precision(reason="fp16 softmax weights"):
                    nc.vector.tensor_copy(raws[h][:, lo : lo + w], pvh[0:64, plo : plo + w])

            def norm_part(h, lo, w, pvh, plo, tag):
                # den (psum) -> sbuf fp32 -> approx reciprocal -> PE
                # broadcast -> multiply raws into oTp
                den = dpool.tile([1, 1024], dt.float32, tag="den", bufs=1, name="den")
                nc.vector.tensor_copy(den[0:1, 0:w], pvh[64:65, plo : plo + w])
                dinv = dpool.tile([1, 1024], dt.float32, tag="dinv", bufs=1, name="dinv")
                nc.vector.reciprocal_approx_fast(out=dinv[0:1, 0:w], in_=den[0:1, 0:w])
                bc = ps.tile([64, 1024], dt.float32, tag=tag, name="bc")
                for d_ in range(w // 512):
                    nc.tensor.matmul(
                        bc[:, d_ * 512 : (d_ + 1) * 512],
                        ones64f[:],
                        dinv[0:1, d_ * 512 : (d_ + 1) * 512],
                        start=True, stop=True,
                    )
                row = 64 * (h % 2)
                with nc.allow_low_precision(reason="fp16 softmax weights"):
                    nc.vector.tensor_tensor(
                        out=oTp[h // 2][row : row + 64, lo : lo + w],
                        in0=raws[h][:, lo : lo + w], in1=bc[:, 0:w], op=ALU.mult,
                    )

            sqh = {}

            def norms_half(h, half):
                if h not in sqh:
                    sqh[h] = u4.tile([128, N], dt.float16, tag="u4", name=f"sqh{h}")
                sq = sqh[h]
                sq_half(h, "q", sq, half)
                sq_half(h, "k", sq, half)
                norm_red(h, "q", sq, half, "pA")
                norm_red(h, "k", sq, half, "pB")

            def yout(pair, ib, tag, act_copy=False):
                yp = ps.tile([128, D], dt.float32, tag=tag, name="yp")
                for fc in range(2):
                    nc.tensor.matmul(
                        yp[:, fc * 512 : (fc + 1) * 512],
                        oTp[pair][:, ib * 128 : (ib + 1) * 128],
                        wotp[pair][:, fc * 512 : (fc + 1) * 512],
                        start=True, stop=True,
                    )
                yac = u4.tile([128, D], dt.float16, tag="u4", name="yac")
                with nc.allow_low_precision(reason="fp16 partial output"):
                    if act_copy:
                        nc.scalar.copy(yac[:], yp[:])
                    else:
                        nc.vector.tensor_copy(yac[:], yp[:])
                nc.sync.dma_start(ydram[pair][ib * 128 : (ib + 1) * 128, :], yac[:])

            # ================= emission schedule =================
            # ---- lead-in: pair-0 q proj + q2; k proj i-half A; first d2 ----
            sqh[0] = u4.tile([128, N], dt.float16, tag="u4", name="sq0")
            # rows 64-127 are read (x0) by the q norm_reds before the k
            # squares land -- uninitialized NaN bits would poison the PE sum
            nc.gpsimd.memset(sqh[0][64:128, :], 0.0)
            for ic in range(2):
                proj_qk_ic(wq_all, 0, q_aug, 0, ic, "pA" if ic % 2 == 0 else "pB")
            sq_half(0, "q", sqh[0], 0)
            for ic in range(2, IC):
                proj_qk_ic(wq_all, 0, q_aug, 0, ic, "pA" if ic % 2 == 0 else "pB")
            sq_half(0, "q", sqh[0], 1)
            norm_red(0, "q", sqh[0], 0, "pA")
            norm_red(0, "q", sqh[0], 1, "pB")
            for ic in range(2):
                proj_qk_ic(wk_all, 2, k_stat, 0, ic, "pA" if ic % 2 == 0 else "pB")
            sq_half(0, "k", sqh[0], 0)
            norm_red(0, "k", sqh[0], 0, "pA")
            pend_st[(0, 0)] = st_halves(0, 0)
            # v / wo loads go behind the q2 DMAs on the SWDGE queue
            for k in range(KB):
                nc.gpsimd.dma_start(
                    wv_all[:, k * HS : (k + 1) * HS], wv[k * 128 : (k + 1) * 128, :]
                )
            for p_ in range(2):
                nc.gpsimd.dma_start(wotp[p_][:], wo[p_ * 128 : (p_ + 1) * 128, :])

            def kproj_late(ic):
                proj_qk_ic(wk_all, 2, k_stat, 0, ic, "pA" if ic % 2 == 0 else "pB")

            def k_stats_b():
                sq_half(0, "k", sqh[0], 1)
                norm_red(0, "k", sqh[0], 1, "pB")

            sqrt_fills = {
                0: [lambda: kproj_late(2), lambda: kproj_late(3), k_stats_b]
                   + [lambda ic=ic: proj_qk_ic(wq_all, 0, q_aug, 1, ic,
                                               "pA" if ic % 2 == 0 else "pB")
                      for ic in range(IC)]
                   + [lambda jb=jb: vp_one(jb) for jb in range(NB)],
                1: [],
                2: [lambda ib=ib: yout(0, ib, "pB" if ib % 2 == 0 else "pA")
                    for ib in range(6)],
                3: [lambda ib=ib: yout(0, ib, "pB" if ib % 2 == 0 else "pA")
                    for ib in range(12, NB)],
            }
            exp_fills = {
                (0, 1): [lambda: proj_qk_ic(wk_all, 2, k_stat, 1, 0, "pA")],
                (0, 2): [lambda: proj_qk_ic(wk_all, 2, k_stat, 1, 1, "pB")],
                (0, 4): [lambda: proj_qk_ic(wk_all, 2, k_stat, 1, 2, "pA")],
                (0, 5): [lambda: proj_qk_ic(wk_all, 2, k_stat, 1, 3, "pB")],
                (0, 6): [lambda: norms_half(1, 0)],
                (0, 7): [lambda: norms_half(1, 1)],
                (1, 1): [lambda: norms_half(2, 0)],
                (1, 2): [lambda: norms_half(2, 1)],
                (1, 4): [lambda: norms_half(3, 0)],
                (1, 6): [lambda: norms_half(3, 1)],
                (2, 1): [lambda: yout(0, 6, "pA")],
                (2, 2): [lambda: yout(0, 7, "pB")],
                (2, 4): [lambda: yout(0, 8, "pA")],
                (2, 5): [lambda: yout(0, 9, "pB")],
                (2, 6): [lambda: yout(0, 10, "pA")],
                (2, 7): [lambda: yout(0, 11, "pB")],
                (3, 4): [lambda: yout(1, 0, "pA"), lambda: yout(1, 1, "pA")],
                (3, 6): [lambda: yout(1, 2, "pA"), lambda: yout(1, 3, "pA")],
                (3, 7): [lambda: yout(1, 4, "pA"), lambda: yout(1, 5, "pA")],
            }

            # ---- per-head phases ----
            for h in range(HPC):
                # ---------- sqrt phase ----------
                fills = iter(sqrt_fills[h])

                def fill(n=1):
                    for _ in range(n):
                        f = next(fills, None)
                        if f is not None:
                            f()

                # B-half normalize of previous head early in this phase
                ts0 = pend_st.pop((h, 0))
                sqrt_psum_direct(h, 0, ts0)
                if h > 0:
                    pvb = pend_pv.pop(h - 1)
                    raw_part(h - 1, 1024, 1024, pvb, 0)
                    norm_part(h - 1, 1024, 1024, pvb, 0, "pA")
                for jb in range(1, NDIR):
                    ts = st_halves(h, jb)
                    sqrt_psum_direct(h, jb, ts)
                    fill(1)
                for g in (NDIR // 4, NDIR // 4 + 1):
                    for jb in range(4 * g, 4 * g + 4):
                        ts = st_halves(h, jb)
                        st_cast(h, jb, ts)
                        fill(1)
                    sqrt_group(g)
                fill(100)

                # ---------- exp phase ----------
                pvA = ps.tile([65, 1024], dt.float32, tag="pA", name="pvA")
                pvB = ps.tile([65, 1024], dt.float32, tag="pB", name="pvB")
                for c in range(NEC):
                    exp_chunk(h, c, pvA if c < 4 else pvB)
                    if c == 3:
                        raw_part(h, 0, 1024, pvA, 0)
                        norm_part(h, 0, 1024, pvA, 0, "pA")
                    if c == 5 and h == 3:
                        # head-3 B first quarter (cols 1024-1535) is final
                        raw_part(3, 1024, 512, pvB, 0)
                        norm_part(3, 1024, 512, pvB, 0, "pA")
                        for ib in range(8, 12):
                            yout(1, ib, "pA")
                    for f in exp_fills.get((h, c), []):
                        f()
                if h < HPC - 1:
                    pend_st[(h + 1, 0)] = st_halves(h + 1, 0)
                    raw_part(h, 1024, 1024, pvB, 0)
                    pend_pv[h] = pvB

            # ---------- tail: head-3 B second quarter, rest of pair-1 y ----
            raw_part(3, 1536, 512, pvB, 512)
            norm_part(3, 1536, 512, pvB, 512, "pB")
            for i_, ib in enumerate(list(range(12, NB)) + [6, 7]):
                yout(1, ib, "pB" if ib % 2 == 0 else "pA", act_copy=(i_ % 2 == 0))

    nc.compile()
    return nc


def _prep_in_maps(x, wq, bq, wk, bk, wv, wo):
    f16 = np.float16
    in_maps = []
    xTs = [np.ascontiguousarray(x[b].T).astype(f16) for b in range(B)]
    for c in range(8):
        b, hg = divmod(c, HPC)
        hs = hg * HS
        biases = np.stack(
            [
                bq[hs : hs + 128],
                bq[hs + 128 : hs + 256],
                -2.0 * bk[hs : hs + 128],
                -2.0 * bk[hs + 128 : hs + 256],
            ],
            axis=1,
        ).astype(np.float32)
        in_maps.append(
            {
                "xT": xTs[b],
                "wq_t": np.ascontiguousarray(wq[hs : hs + HS, :].T).astype(f16),
                "wk_t": np.ascontiguousarray(-2.0 * wk[hs : hs + HS, :].T).astype(f16),
                "wv_t": np.ascontiguousarray(wv[hs : hs + HS, :].T).astype(f16),
                "woT": np.ascontiguousarray(wo[:, hs : hs + HS].T).astype(f16),
                "biases": np.ascontiguousarray(biases),
            }
        )
    return in_maps


def _get_nc():
    if "nc" not in _CACHE:
        _CACHE["nc"] = _build()
    return _CACHE["nc"]


def run(inputs, trace=False, **trace_kwargs):
    """Run on 8 cores; returns (full_output, BassKernelResults)."""
    from concourse.bass_utils import run_bass_kernel_spmd

    nc = _get_nc()
    wv_np = np.asarray(inputs["wv"], np.float32)
    bv_np = np.asarray(inputs["bv"], np.float32)
    wo_np = np.asarray(inputs["wo"], np.float32)
    in_maps = _prep_in_maps(
        np.asarray(inputs["x"], np.float32),
        np.asarray(inputs["wq"], np.float32), np.asarray(inputs["bq"], np.float32),
        np.asarray(inputs["wk"], np.float32), np.asarray(inputs["bk"], np.float32),
        wv_np, wo_np,
    )
    res = run_bass_kernel_spmd(nc, in_maps, list(range(8)), trace=trace, **trace_kwargs)
    # v-bias folds to wo @ bv after softmax normalization
    bo_eff = np.asarray(inputs["bo"], np.float32) + wo_np @ bv_np
    out = np.empty((B, N, D), np.float32)
    for b in range(B):
        acc = res.results[b * HPC]["y0"].astype(np.float32)
        acc = acc + res.results[b * HPC]["y1"].astype(np.float32)
        for c in range(b * HPC + 1, (b + 1) * HPC):
            acc = acc + res.results[c]["y0"].astype(np.float32)
            acc = acc + res.results[c]["y1"].astype(np.float32)
        out[b] = acc + bo_eff
    return out, res


def kernel(**inputs) -> np.ndarray:
    out, _ = run(inputs, trace=False)
    return out


if __name__ == "__main__":
    rng = np.random.default_rng(0)
    ins = {
        "x": rng.standard_normal((B, N, D)).astype(np.float32),
        "wq": (rng.standard_normal((D, D)) * 0.02).astype(np.float32),
        "bq": (rng.standard_normal(D) * 0.02).astype(np.float32),
        "wk": (rng.standard_normal((D, D)) * 0.02).astype(np.float32),
        "bk": (rng.standard_normal(D) * 0.02).astype(np.float32),
        "wv": (rng.standard_normal((D, D)) * 0.02).astype(np.float32),
        "bv": (rng.standard_normal(D) * 0.02).astype(np.float32),
        "wo": (rng.standard_normal((D, D)) * 0.02).astype(np.float32),
        "bo": (rng.standard_normal(D) * 0.02).astype(np.float32),
    }
    print(kernel(**ins).shape)


# revision 43
# speedup vs baseline: 1.1955x; 1.0134x over previous
"""L2-distance self-attention (B=2, N=2048, D=1024, H=16) on 8 trn2 NeuronCores.

Sharding: core c handles batch c//4 and heads 4*(c%4) .. 4*(c%4)+4.
Each core computes its 4 heads end-to-end and returns TWO (2048, 1024) fp16
partials of the output projection (head pair 0 and head pair 1); the host
sums the 8 partials per batch and adds bo_eff = bo + wo @ bv (the v-bias
contributes exactly wo@bv after softmax normalization, so it is folded out
of the device kernel).

Layout: q_aug rows = [qb(0-63); ones(64); q2(65)], k_stat rows =
[kb2(0-63); k2(64); ones(65)] so one K=66 matmul emits the full
d2[j,i] = q2[i] + k2[j] - 2 q.k.  Row 65 of q_aug is written by an
SBUF->SBUF DMA (engines cannot address single partitions above 64, DMA
can).  kb2 = -2*(x wk + bk) is host-prescaled via wk/bk.

Per-head pipeline (ACT is the bottleneck at ~64us/head):
  sqrt phase: j-blocks 0-7 are consumed by ACT Sqrt straight from PSUM
    (two [128,1024] half-ops per block); blocks 8-15 are drained by DVE
    copies into the s tile and ACT runs Sqrt in-place over two 4-block
    groups.  This splits the drain work between ACT and DVE so neither
    paces the other.
  exp phase: ACT Exp (scale=-1) over strided i-chunks; PE runs PV
    matmuls (v_aug ones column -> row 64 = softmax denominator) plus the
    first j-block of the NEXT head's d2 so the next sqrt phase starts hot.
  normalize: denominator -> fp32 -> reciprocal_approx_fast (from SBUF,
    not PSUM -- the custom DVE op misreads PSUM on HW), PE broadcast
    matmul, DVE multiply into oTp; done per i-half in DVE-idle exp phases
    (head 3's tail half in i-quarters to shorten the tail).
  out-proj: pair-0 y during heads 2/3 sqrt phases, pair-1 y overlapping
    exp(3) and the tail; separate DRAM tensor per pair.
"""

import sys

for p in ("/opt/trn_rl_repo", "/root/.axon_site/_ro/trn_rl_repo"):
    if p not in sys.path:
        sys.path.append(p)

import numpy as np

B, N, D, H = 2, 2048, 1024, 16
HD = 64          # head dim
HPC = 4          # heads per core
HS = HPC * HD    # head-group width per core (256)
NB = N // 128    # 16 j-blocks
IC = N // 512    # 4 i-chunks of 512
KB = D // 128    # 8 contraction blocks for projections
EC = 256         # exp/PV i-chunk width
NEC = N // EC    # 8 exp chunks per head
NDIR = 8         # j-blocks consumed psum-direct by ACT (rest DVE-drained)

_CACHE = {}


def _build():
    import concourse.bacc as bacc
    import concourse.mybir as mybir
    import concourse.tile as tile

    dt = mybir.dt
    AF = mybir.ActivationFunctionType
    ALU = mybir.AluOpType

    nc = bacc.Bacc("TRN2", target_bir_lowering=False, debug=False)

    # ---- DRAM I/O (per core) ----
    xT = nc.dram_tensor("xT", [D, N], dt.float16, kind="ExternalInput")
    wq = nc.dram_tensor("wq_t", [D, HS], dt.float16, kind="ExternalInput")
    wk = nc.dram_tensor("wk_t", [D, HS], dt.float16, kind="ExternalInput")
    wv = nc.dram_tensor("wv_t", [D, HS], dt.float16, kind="ExternalInput")
    wo = nc.dram_tensor("woT", [HS, D], dt.float16, kind="ExternalInput")
    bias_d = nc.dram_tensor("biases", [128, 4], dt.float32, kind="ExternalInput")
    y0 = nc.dram_tensor("y0", [N, D], dt.float16, kind="ExternalOutput")
    y1 = nc.dram_tensor("y1", [N, D], dt.float16, kind="ExternalOutput")
    ydram = [y0, y1]

    with tile.TileContext(nc) as tc:
        with (
            tc.tile_pool(name="cst", bufs=1) as cst,
            tc.tile_pool(name="u4", bufs=9) as u4,        # 4KB slots: xt, sq, yac
            tc.tile_pool(name="wp", bufs=1) as wp,
            tc.tile_pool(name="wop", bufs=1) as wop,
            tc.tile_pool(name="aug", bufs=1) as aug,
            tc.tile_pool(name="rawp", bufs=2) as rawp,    # raws[h] rotate
            tc.tile_pool(name="dp", bufs=1) as dpool,
            tc.tile_pool(name="spool", bufs=1) as spool,
            tc.tile_pool(name="e8", bufs=2) as e8,
            tc.tile_pool(name="psum", bufs=2, space="PSUM") as ps,
        ):
            # ---- constants ----
            ones_row = cst.tile([1, 512], dt.float16, tag="ones_row")
            nc.gpsimd.memset(ones_row[:], 1.0)
            ones64f = cst.tile([1, 64], dt.float32, tag="ones64f")
            nc.gpsimd.memset(ones64f[:], 1.0)
            # norm reduce matrix: col0 = 1 on rows 0-63 (q2 = sum qb^2),
            # col32 = 0.25 on rows 64-127 (k2 = 0.25*sum kb2^2)
            emat = cst.tile([128, 33], dt.float16, tag="emat")
            nc.gpsimd.memset(emat[:], 0.0)
            nc.gpsimd.memset(emat[0:64, 0:1], 1.0)
            nc.gpsimd.memset(emat[64:128, 32:33], 0.25)

            bias_pp = cst.tile([128, 4], dt.float32, tag="bias_pp")
            nc.sync.dma_start(bias_pp[:], bias_d[:, :])
            # exp shift: e' = exp(5 - s); cancels in the softmax normalize
            bias5 = cst.tile([128, 1], dt.float32, tag="bias5")
            nc.gpsimd.memset(bias5[:], 5.0)

            # ---- per-head tiles ----
            q_aug = [aug.tile([66, N], dt.float16, tag=f"qa{h}", name=f"qa{h}") for h in range(HPC)]
            k_stat = [aug.tile([66, N], dt.float16, tag=f"ks{h}", name=f"ks{h}") for h in range(HPC)]
            for h in range(HPC):
                # q_aug row 64 = ones (const); row 65 overwritten with q2 by
                # DMA.  k_stat row 65 = ones (const); row 64 overwritten
                # with k2 by a DVE copy (base-64 is engine-addressable).
                nc.gpsimd.memset(q_aug[h][64:66, :], 1.0)
                nc.gpsimd.memset(k_stat[h][64:66, :], 1.0)
            # v per j-block: [p, h(4), d(65)]; d 64 = ones column -> PV row
            # 64 = softmax denominator
            v_jb = [aug.tile([128, HPC * 65], dt.float16, tag=f"v{jb}", name=f"v{jb}") for jb in range(NB)]
            for jb in range(NB):
                nc.gpsimd.memset(
                    v_jb[jb][:].rearrange("p (b d) -> p b d", d=65)[:, :, 64:65], 1.0
                )
            oTp = [
                aug.tile([128, N], dt.float16, tag="oTp0", name="oTp0"),
                aug.tile([128, N], dt.float16, tag="oTp1", name="oTp1"),
            ]

            # PE warmup: dependency-free matmuls release the HAM clock gate
            wup = ps.tile([128, 512], dt.float32, tag="pA", name="wup")
            for r in range(12):
                nc.tensor.matmul(
                    wup[:], ones_row[0:1, 0:128], ones_row[0:1, :],
                    start=(r == 0), stop=(r == 11),
                )

            # ---- input DMA ----
            xt = [u4.tile([128, N], dt.float16, tag="u4", name=f"xt{k}") for k in range(KB)]
            wq_all = wp.tile([128, KB * HS], dt.float16, tag="wq_all")
            wk_all = wp.tile([128, KB * HS], dt.float16, tag="wk_all")
            wv_all = wp.tile([128, KB * HS], dt.float16, tag="wv_all")
            for k in range(KB):
                nc.sync.dma_start(
                    wq_all[:, k * HS : (k + 1) * HS], wq[k * 128 : (k + 1) * 128, :]
                )
                nc.sync.dma_start(xt[k][:], xT[k * 128 : (k + 1) * 128, :])
            for k in range(KB):
                nc.sync.dma_start(
                    wk_all[:, k * HS : (k + 1) * HS], wk[k * 128 : (k + 1) * 128, :]
                )
            wotp = [wop.tile([128, D], dt.float16, tag=f"wop{p}", name=f"wop{p}") for p in range(2)]

            # ---- big SBUF tiles ----
            s = spool.tile([128, NB * N], dt.float16, tag="s")
            sv = s[:].rearrange("p (t i) -> p t i", t=NB)

            raws = [None] * HPC
            pend_st = {}   # (h, jb) -> (tileA, tileB) kept in PSUM
            pend_pv = {}   # h -> pvB psum tile (den row consumed next phase)

            ic_sl = lambda ic: slice(ic * 512, (ic + 1) * 512)

            # ---- helpers (emission) ----
            def proj_qk_half(w_all, bcol, dest, m, half, tag):
                # heads 2m, 2m+1; psum (128 d, 1024 i); bias fused in copy
                lo = half * 1024
                p = ps.tile([128, 1024], dt.float32, tag=tag, name="pp")
                for d_ in range(2):
                    for k in range(KB):
                        nc.tensor.matmul(
                            p[:, d_ * 512 : (d_ + 1) * 512],
                            w_all[:, k * HS + m * 128 : k * HS + (m + 1) * 128],
                            xt[k][:, lo + d_ * 512 : lo + (d_ + 1) * 512],
                            start=(k == 0), stop=(k == KB - 1),
                        )
                for hf in range(2):
                    with nc.allow_low_precision(reason="fp16 activations"):
                        nc.vector.tensor_scalar_add(
                            out=dest[2 * m + hf][0:64, lo : lo + 1024],
                            in0=p[64 * hf : 64 * hf + 64, :],
                            scalar1=bias_pp[64 * hf : 64 * hf + 64, bcol + m : bcol + m + 1],
                        )

            def sq_half(h, which, sq, half):
                # squares for one i-half (1024 cols)
                lo = half * 1024
                src = q_aug[h] if which == "q" else k_stat[h]
                r0 = 0 if which == "q" else 64
                nc.vector.tensor_tensor(
                    out=sq[r0 : r0 + 64, lo : lo + 1024],
                    in0=src[0:64, lo : lo + 1024], in1=src[0:64, lo : lo + 1024],
                    op=ALU.mult,
                )

            def norm_red(h, which, sq, half, tag):
                # q2 (row 0) and k2 (row 32) of psum; q2 -> stage -> DMA to
                # q_aug row 65, k2 -> k_stat row 64 (direct DVE copy)
                lo = half * 1024
                p = ps.tile([33, 1024], dt.float32, tag=tag, name="np")
                for d_ in range(2):
                    ic = 2 * half + d_
                    nc.tensor.matmul(
                        p[:, d_ * 512 : (d_ + 1) * 512], emat[:], sq[:, ic_sl(ic)],
                        start=True, stop=True,
                    )
                with nc.allow_low_precision(reason="fp16 stats"):
                    if which == "q":
                        st_ = dpool.tile([1, 1024], dt.float16, tag="q2st", bufs=2, name="q2st")
                        nc.vector.tensor_copy(st_[:], p[0:1, :])
                        nc.gpsimd.dma_start(q_aug[h][65:66, lo : lo + 1024], st_[:])
                    else:
                        nc.vector.tensor_copy(k_stat[h][64:65, lo : lo + 1024], p[32:33, :])

            def vp_one(jb):
                p = ps.tile([128, HS], dt.float32, tag="pA" if jb % 2 == 0 else "pB", name="vp")
                for k in range(KB):
                    nc.tensor.matmul(
                        p[:], xt[k][:, jb * 128 : (jb + 1) * 128],
                        wv_all[:, k * HS : (k + 1) * HS],
                        start=(k == 0), stop=(k == KB - 1),
                    )
                dst = v_jb[jb][:].rearrange("p (h d) -> p h d", d=65)[:, :, 0:64]
                nc.vector.tensor_copy(dst, p[:].rearrange("p (h d) -> p h d", d=64))

            def st_halves(h, jb):
                # full d2 for j-block jb, both i-halves, kept in PSUM
                ts = []
                for half, tag in ((0, "pA"), (1, "pB")):
                    t_ = ps.tile([128, 1024], dt.float32, tag=tag, name=f"st{half}")
                    for d_ in range(2):
                        nc.tensor.matmul(
                            t_[:, d_ * 512 : (d_ + 1) * 512],
                            k_stat[h][0:66, jb * 128 : (jb + 1) * 128],
                            q_aug[h][0:66, half * 1024 + d_ * 512 : half * 1024 + (d_ + 1) * 512],
                            start=True, stop=True,
                        )
                    ts.append(t_)
                return ts

            def st_cast(h, jb, ts):
                # drain d2 psum -> s (fp16); sqrt later runs in-place
                for half in (0, 1):
                    with nc.allow_low_precision(reason="fp16 d2"):
                        nc.vector.tensor_copy(
                            s[:, jb * N + half * 1024 : jb * N + (half + 1) * 1024],
                            ts[half][:],
                        )

            def sqrt_psum_direct(h, jb, ts):
                for half in (0, 1):
                    nc.scalar.activation(
                        s[:, jb * N + half * 1024 : jb * N + (half + 1) * 1024],
                        ts[half][:], AF.Sqrt,
                    )

            def sqrt_group(g):
                lo, hi = 4 * g * N, 4 * (g + 1) * N
                nc.scalar.activation(s[:, lo:hi], s[:, lo:hi], AF.Sqrt)

            def exp_act(h, c):
                e = e8.tile([128, NB * EC], dt.float16, tag="e8", name="e")
                nc.scalar.activation(
                    e[:].rearrange("p (t i) -> p t i", t=NB),
                    sv[:, :, c * EC : (c + 1) * EC],
                    AF.Exp, scale=-1.0,
                )
                return e

            def pv_mms(h, c, pvh, e=None):
                cc = (c % 4) * EC
                for t in range(NB):
                    nc.tensor.matmul(
                        pvh[:, cc : cc + EC],
                        v_jb[t][:, h * 65 : h * 65 + 65],
                        e[:, t * EC : (t + 1) * EC],
                        start=(t == 0), stop=(t == NB - 1),
                    )

            def exp_chunk(h, c, pvh):
                e = exp_act(h, c)
                pv_mms(h, c, pvh, e)

            def raw_part(h, lo, w, pvh, plo):
                # pv rows 0-63 -> raws fp16 (row 64 = den stays in psum)
                if lo == 0:
                    raws[h] = rawp.tile([64, N], dt.float16, tag="raw", name=f"raw{h}")
                with nc.allow_low_precision(reason="fp16 softmax weights"):
                    nc.vector.tensor_copy(raws[h][:, lo : lo + w], pvh[0:64, plo : plo + w])

            def norm_part(h, lo, w, pvh, plo, tag):
                # den (psum) -> sbuf fp32 -> approx reciprocal -> PE
                # broadcast -> multiply raws into oTp
                den = dpool.tile([1, 1024], dt.float32, tag="den", bufs=1, name="den")
                nc.vector.tensor_copy(den[0:1, 0:w], pvh[64:65, plo : plo + w])
                dinv = dpool.tile([1, 1024], dt.float32, tag="dinv", bufs=1, name="dinv")
                nc.vector.reciprocal_approx_fast(out=dinv[0:1, 0:w], in_=den[0:1, 0:w])
                bc = ps.tile([64, 1024], dt.float32, tag=tag, name="bc")
                for d_ in range(w // 512):
                    nc.tensor.matmul(
                        bc[:, d_ * 512 : (d_ + 1) * 512],
                        ones64f[:],
                        dinv[0:1, d_ * 512 : (d_ + 1) * 512],
                        start=True, stop=True,
                    )
                row = 64 * (h % 2)
                with nc.allow_low_precision(reason="fp16 softmax weights"):
                    nc.vector.tensor_tensor(
                        out=oTp[h // 2][row : row + 64, lo : lo + w],
                        in0=raws[h][:, lo : lo + w], in1=bc[:, 0:w], op=ALU.mult,
                    )

            sqh = {}

            def norms_half(h, half):
                if h not in sqh:
                    sqh[h] = u4.tile([128, N], dt.float16, tag="u4", name=f"sqh{h}")
                sq = sqh[h]
                sq_half(h, "q", sq, half)
                sq_half(h, "k", sq, half)
                norm_red(h, "q", sq, half, "pA")
                norm_red(h, "k", sq, half, "pB")

            def yout(pair, ib, tag, act_copy=False):
                yp = ps.tile([128, D], dt.float32, tag=tag, name="yp")
                for fc in range(2):
                    nc.tensor.matmul(
                        yp[:, fc * 512 : (fc + 1) * 512],
                        oTp[pair][:, ib * 128 : (ib + 1) * 128],
                        wotp[pair][:, fc * 512 : (fc + 1) * 512],
                        start=True, stop=True,
                    )
                yac = u4.tile([128, D], dt.float16, tag="u4", name="yac")
                with nc.allow_low_precision(reason="fp16 partial output"):
                    if act_copy:
                        nc.scalar.copy(yac[:], yp[:])
                    else:
                        nc.vector.tensor_copy(yac[:], yp[:])
                nc.sync.dma_start(ydram[pair][ib * 128 : (ib + 1) * 128, :], yac[:])

            # ================= emission schedule =================
            # ---- lead-in: pair-0 q proj + q2; k proj i-half A; first d2 ----
            sqh[0] = u4.tile([128, N], dt.float16, tag="u4", name="sq0")
            # rows 64-127 are read (x0) by the q norm_reds before the k
            # squares land -- uninitialized NaN bits would poison the PE sum
            nc.gpsimd.memset(sqh[0][64:128, :], 0.0)
            # i-half A chain first so the first d2 block starts ASAP
            proj_qk_half(wq_all, 0, q_aug, 0, 0, "pA")
            sq_half(0, "q", sqh[0], 0)
            proj_qk_half(wk_all, 2, k_stat, 0, 0, "pB")
            sq_half(0, "k", sqh[0], 0)
            norm_red(0, "q", sqh[0], 0, "pA")
            norm_red(0, "k", sqh[0], 0, "pB")
            proj_qk_half(wq_all, 0, q_aug, 0, 1, "pA")
            sq_half(0, "q", sqh[0], 1)
            norm_red(0, "q", sqh[0], 1, "pB")
            pend_st[(0, 0)] = st_halves(0, 0)
            # v / wo loads go behind the q2 DMAs on the SWDGE queue
            for k in range(KB):
                nc.gpsimd.dma_start(
                    wv_all[:, k * HS : (k + 1) * HS], wv[k * 128 : (k + 1) * 128, :]
                )
            for p_ in range(2):
                nc.gpsimd.dma_start(wotp[p_][:], wo[p_ * 128 : (p_ + 1) * 128, :])

            def k_stats_b():
                sq_half(0, "k", sqh[0], 1)
                norm_red(0, "k", sqh[0], 1, "pB")

            sqrt_fills = {
                0: [lambda: proj_qk_half(wk_all, 2, k_stat, 0, 1, "pA"), k_stats_b,
                    lambda: proj_qk_half(wq_all, 0, q_aug, 1, 0, "pB"),
                    lambda: proj_qk_half(wq_all, 0, q_aug, 1, 1, "pA"),
                    lambda: proj_qk_half(wk_all, 2, k_stat, 1, 0, "pB"),
                    lambda: proj_qk_half(wk_all, 2, k_stat, 1, 1, "pA")]
                   + [lambda jb=jb: vp_one(jb) for jb in range(NB)],
                1: [],
                2: [lambda ib=ib: yout(0, ib, "pB" if ib % 2 == 0 else "pA")
                    for ib in range(6)],
                3: [lambda ib=ib: yout(0, ib, "pB" if ib % 2 == 0 else "pA")
                    for ib in range(12, NB)],
            }
            exp_fills = {
                (0, 2): [lambda: norms_half(1, 0)],
                (0, 5): [lambda: norms_half(1, 1)],
                (1, 1): [lambda: norms_half(2, 0)],
                (1, 2): [lambda: norms_half(2, 1)],
                (1, 4): [lambda: norms_half(3, 0)],
                (1, 6): [lambda: norms_half(3, 1)],
                (2, 1): [lambda: yout(0, 6, "pA")],
                (2, 2): [lambda: yout(0, 7, "pB")],
                (2, 4): [lambda: yout(0, 8, "pA")],
                (2, 5): [lambda: yout(0, 9, "pB")],
                (2, 6): [lambda: yout(0, 10, "pA")],
                (2, 7): [lambda: yout(0, 11, "pB")],
                (3, 4): [lambda: yout(1, 0, "pA"), lambda: yout(1, 1, "pA")],
                (3, 6): [lambda: yout(1, 2, "pA"), lambda: yout(1, 3, "pA")],
                (3, 7): [lambda: yout(1, 4, "pA"), lambda: yout(1, 5, "pA")],
            }

            # ---- per-head phases ----
            for h in range(HPC):
                # ---------- sqrt phase ----------
                fills = iter(sqrt_fills[h])

                def fill(n=1):
                    for _ in range(n):
                        f = next(fills, None)
                        if f is not None:
                            f()

                # B-half normalize of previous head early in this phase
                ts0 = pend_st.pop((h, 0))
                sqrt_psum_direct(h, 0, ts0)
                if h > 0:
                    pvb = pend_pv.pop(h - 1)
                    raw_part(h - 1, 1024, 1024, pvb, 0)
                    norm_part(h - 1, 1024, 1024, pvb, 0, "pA")
                # PE order interleaves drain-block d2 with direct-block d2 so
                # the DVE drains spread over the whole phase and the group
                # sqrts fire right after the psum-direct ones.  h==0 delays
                # the drains until the pair-0 k i-half-B stats fills land.
                dstart = 3 if h == 0 else 1
                dj_iter = iter(range(NDIR, NB))
                for jb in range(1, NDIR):
                    ts = st_halves(h, jb)
                    sqrt_psum_direct(h, jb, ts)
                    fill(1)
                    if jb >= dstart:
                        dj = next(dj_iter, None)
                        if dj is not None:
                            st_cast(h, dj, st_halves(h, dj))
                for dj in dj_iter:
                    st_cast(h, dj, st_halves(h, dj))
                    fill(1)
                for g in range(NDIR // 4, 4):
                    sqrt_group(g)
                fill(100)

                # ---------- exp phase ----------
                pvA = ps.tile([65, 1024], dt.float32, tag="pA", name="pvA")
                pvB = ps.tile([65, 1024], dt.float32, tag="pB", name="pvB")
                for c in range(NEC):
                    if c == NEC - 1 and h < HPC - 1:
                        # next head's first d2 block lands inside the final
                        # exp window (ahead of PV(7) in the PE queue)
                        e_last = exp_act(h, c)
                        pend_st[(h + 1, 0)] = st_halves(h + 1, 0)
                        pv_mms(h, c, pvB, e_last)
                    else:
                        exp_chunk(h, c, pvA if c < 4 else pvB)
                    if c == 3:
                        raw_part(h, 0, 1024, pvA, 0)
                        norm_part(h, 0, 1024, pvA, 0, "pA")
                    if c == 5 and h == 3:
                        # head-3 B first quarter (cols 1024-1535) is final
                        raw_part(3, 1024, 512, pvB, 0)
                        norm_part(3, 1024, 512, pvB, 0, "pA")
                        for ib in range(8, 12):
                            yout(1, ib, "pA")
                    for f in exp_fills.get((h, c), []):
                        f()
                if h < HPC - 1:
                    raw_part(h, 1024, 1024, pvB, 0)
                    pend_pv[h] = pvB

            # ---------- tail: head-3 B second quarter, rest of pair-1 y ----
            raw_part(3, 1536, 512, pvB, 512)
            norm_part(3, 1536, 512, pvB, 512, "pB")
            for i_, ib in enumerate(list(range(12, NB)) + [6, 7]):
                yout(1, ib, "pB" if ib % 2 == 0 else "pA", act_copy=(i_ % 2 == 0))

    nc.compile()
    return nc


def _prep_in_maps(x, wq, bq, wk, bk, wv, wo):
    f16 = np.float16
    in_maps = []
    xTs = [np.ascontiguousarray(x[b].T).astype(f16) for b in range(B)]
    for c in range(8):
        b, hg = divmod(c, HPC)
        hs = hg * HS
        biases = np.stack(
            [
                bq[hs : hs + 128],
                bq[hs + 128 : hs + 256],
                -2.0 * bk[hs : hs + 128],
                -2.0 * bk[hs + 128 : hs + 256],
            ],
            axis=1,
        ).astype(np.float32)
        in_maps.append(
            {
                "xT": xTs[b],
                "wq_t": np.ascontiguousarray(wq[hs : hs + HS, :].T).astype(f16),
                "wk_t": np.ascontiguousarray(-2.0 * wk[hs : hs + HS, :].T).astype(f16),
                "wv_t": np.ascontiguousarray(wv[hs : hs + HS, :].T).astype(f16),
                "woT": np.ascontiguousarray(wo[:, hs : hs + HS].T).astype(f16),
                "biases": np.ascontiguousarray(biases),
            }
        )
    return in_maps


def _get_nc():
    if "nc" not in _CACHE:
        _CACHE["nc"] = _build()
    return _CACHE["nc"]


def run(inputs, trace=False, **trace_kwargs):
    """Run on 8 cores; returns (full_output, BassKernelResults)."""
    from concourse.bass_utils import run_bass_kernel_spmd

    nc = _get_nc()
    wv_np = np.asarray(inputs["wv"], np.float32)
    bv_np = np.asarray(inputs["bv"], np.float32)
    wo_np = np.asarray(inputs["wo"], np.float32)
    in_maps = _prep_in_maps(
        np.asarray(inputs["x"], np.float32),
        np.asarray(inputs["wq"], np.float32), np.asarray(inputs["bq"], np.float32),
        np.asarray(inputs["wk"], np.float32), np.asarray(inputs["bk"], np.float32),
        wv_np, wo_np,
    )
    res = run_bass_kernel_spmd(nc, in_maps, list(range(8)), trace=trace, **trace_kwargs)
    # v-bias folds to wo @ bv after softmax normalization
    bo_eff = np.asarray(inputs["bo"], np.float32) + wo_np @ bv_np
    out = np.empty((B, N, D), np.float32)
    for b in range(B):
        acc = res.results[b * HPC]["y0"].astype(np.float32)
        acc = acc + res.results[b * HPC]["y1"].astype(np.float32)
        for c in range(b * HPC + 1, (b + 1) * HPC):
            acc = acc + res.results[c]["y0"].astype(np.float32)
            acc = acc + res.results[c]["y1"].astype(np.float32)
        out[b] = acc + bo_eff
    return out, res


def kernel(**inputs) -> np.ndarray:
    out, _ = run(inputs, trace=False)
    return out


if __name__ == "__main__":
    rng = np.random.default_rng(0)
    ins = {
        "x": rng.standard_normal((B, N, D)).astype(np.float32),
        "wq": (rng.standard_normal((D, D)) * 0.02).astype(np.float32),
        "bq": (rng.standard_normal(D) * 0.02).astype(np.float32),
        "wk": (rng.standard_normal((D, D)) * 0.02).astype(np.float32),
        "bk": (rng.standard_normal(D) * 0.02).astype(np.float32),
        "wv": (rng.standard_normal((D, D)) * 0.02).astype(np.float32),
        "bv": (rng.standard_normal(D) * 0.02).astype(np.float32),
        "wo": (rng.standard_normal((D, D)) * 0.02).astype(np.float32),
        "bo": (rng.standard_normal(D) * 0.02).astype(np.float32),
    }
    print(kernel(**ins).shape)
